# revision 1
# baseline (speedup 1.0000x reference)
"""PIoU (pixel-wise IoU) pairwise matrix kernel for Trainium2, 8 NeuronCores.

Math: for each pair (predicted box n, target box m) the reference samples a
16x16 grid of the joint AABB and evaluates a soft membership
F = sigmoid(k(w/2-|A|)) * sigmoid(k(h/2-|B|)) per box, where (A, B) are the
pixel offsets rotated into the box frame.  Both A and B are *affine* in the
grid coordinates (ug, uh), so the sigmoid arguments (s/2 -+ A) for all
256 pixels x 4 fields x {P,Q} come from one K=32 matmul set per (n, 128-m)
tile against a constant basis, with the slope k folded into the activation's
free scale field.

Since P + Q = s >= 8 and k = 10, the larger of sigmoid(kP), sigmoid(kQ) is
>= sigmoid(40) = 1 - 4e-18, so sigmoid(k*min(P,Q)) == sigmoid(kP)*sigmoid(kQ)
to machine precision -- the membership factor is a plain product, no min
needed (a VE min would need two PSUM reads, which the HW forbids).

Structure: a prologue computes the coefficient slabs C for all 4 m-chunks on
DVE; then per m-chunk the lhsT transposes (4 n per PE transpose, 32-row PE
bands via matmul tile_position, ACT casts to fp16) interleave with the main
pipeline per (j, n):
    PE   : 4 fp16 x bf16 matmuls [32,128]x[32,512] -> PSUM [128, 2048] (P|Q)
    ACT  : sig = Sigmoid(K * PQ)   [128, 2048] bf16 (one instruction)
    DVE  : Gm  = sigP * sigQ       [128, 1024] (field memberships)
    DVE  : Fp  = gA * gB -> reduce -> S ; F12 = F1 * F2 -> reduce -> I
NOTE: tensor_tensor_reduce hangs TRN2 hardware -- use mul + reduce only.
Measured (NTFF): DVE is the saturated engine; ACT sigmoid is the floor just
below it; PE matmul rate adapts (p-state) to the pipeline pace.

Sharding: N (predicted) axis split 8 ways; each core computes a [512m, 64n]
slab (output transposed on host).  Dispatch uses a persistent jitted
shard_map callable so steady-state calls skip jax re-trace/re-lowering.
"""

import numpy as np

N = 512
M = 512
G = 16
NPIX = G * G
K_SLOPE = np.float32(10.0)
EPS = np.float32(1e-6)
NC = 8
NLOC = N // NC  # 64 predicted boxes per core
NCHUNK = 4  # m-chunks of 128
KR = 32  # coefficient rows (24 used + 8 zero pad, for 32-aligned PE row bands)

_cache = {}

_QORDER = ("x0", "x1", "y0", "y1", "cx", "cy", "ct", "st", "shw", "shh")


def _derived(b):
    # b: [K,5] float32 -> per-box derived quantities (all float32)
    cx, cy, w, h, t = (b[:, i].astype(np.float32) for i in range(5))
    c, s = np.cos(t).astype(np.float32), np.sin(t).astype(np.float32)
    hw = np.float32(0.5) * (w * np.abs(c) + h * np.abs(s))
    hh = np.float32(0.5) * (w * np.abs(s) + h * np.abs(c))
    return dict(
        cx=cx, cy=cy, ct=c, st=s,
        shw=np.float32(0.5) * w, shh=np.float32(0.5) * h,
        x0=cx - hw, x1=cx + hw, y0=cy - hh, y1=cy + hh,
    )


def _basis():
    # [128, 2048] bf16 (values exact): four replicated 32-row blocks (one per
    # PE row band).  Within a block: P cols 0..1023, Q cols 1024..2047; field
    # f at cols f*256..(f+1)*256 uses rows 3f..3f+2 (P) / 12+3f.. (Q)
    # = (1, Ug, Uh).
    u = (np.arange(G, dtype=np.float32) + np.float32(0.5)) / np.float32(G)
    Ug = np.tile(u, G)      # pixel p = h*G+g -> u[g]
    Uh = np.repeat(u, G)    # -> u[h]
    bas = np.zeros((KR, 8 * NPIX), dtype=np.float32)
    for f in range(4):
        for blk, r0 in ((0, 0), (1, 12)):
            c0 = blk * 4 * NPIX + f * NPIX
            bas[r0 + 3 * f + 0, c0:c0 + NPIX] = 1.0
            bas[r0 + 3 * f + 1, c0:c0 + NPIX] = Ug
            bas[r0 + 3 * f + 2, c0:c0 + NPIX] = Uh
    import ml_dtypes

    return np.vstack([bas] * 4).astype(ml_dtypes.bfloat16)  # [128, 2048]


def _host_constants(loc_p, loc_t):
    """Build per-core input arrays (all O(N+M) host work)."""
    T = _derived(loc_t)
    # TQ [128, 4 chunks, 10]: per-target quantities, m = j*128 + partition
    TQ = np.empty((128, NCHUNK, len(_QORDER)), dtype=np.float32)
    for qi, q in enumerate(_QORDER):
        TQ[:, :, qi] = T[q].reshape(NCHUNK, 128).T

    P = _derived(loc_p)
    PBs = []
    for c in range(NC):
        sl = slice(c * NLOC, (c + 1) * NLOC)
        pb = np.stack([P[q][sl] for q in _QORDER], axis=0)  # [10, 64]
        PBs.append(np.broadcast_to(pb.reshape(1, 10 * NLOC), (128, 10 * NLOC)).copy())
    return _basis(), TQ.reshape(128, NCHUNK * len(_QORDER)), PBs


def _build_nc():
    from contextlib import ExitStack

    import concourse.bacc as bacc
    import concourse.tile as tile
    from concourse import mybir
    from concourse.masks import make_identity

    dt = mybir.dt
    op = mybir.AluOpType
    AF = mybir.ActivationFunctionType
    K = float(K_SLOPE)

    # Bacc (not raw Bass): its finalize() runs generate_event_semaphores,
    # which legalizes Tile's multi-wait sync_info down to <=1 wait per
    # hardware instruction.
    nc = bacc.Bacc(None, target_bir_lowering=False)
    PB_d = nc.declare_dram_parameter("PB", [128, 10 * NLOC], dt.float32, isOutput=False)
    TQ_d = nc.declare_dram_parameter("TQ", [128, NCHUNK * 10], dt.float32, isOutput=False)
    BAS_d = nc.declare_dram_parameter("BASIS", [128, 8 * NPIX], dt.bfloat16, isOutput=False)
    OUT_d = nc.declare_dram_parameter("OUT", [M, NLOC], dt.float32, isOutput=True)

    with tile.TileContext(nc) as tc, ExitStack() as ctx:
        consts = ctx.enter_context(tc.tile_pool(name="consts", bufs=1))
        vminp = ctx.enter_context(tc.tile_pool(name="vminp", bufs=4))
        sigp = ctx.enter_context(tc.tile_pool(name="sigp", bufs=4))
        fpp = ctx.enter_context(tc.tile_pool(name="fpp", bufs=3))
        accp = ctx.enter_context(tc.tile_pool(name="accp", bufs=2))
        psum = ctx.enter_context(tc.tile_pool(name="psum", bufs=2, space="PSUM"))

        ident = consts.tile([128, 128], dt.float32)
        make_identity(nc, ident[:])
        PB = consts.tile([128, 10, NLOC], dt.float32)
        nc.sync.dma_start(out=PB[:].rearrange("p a b -> p (a b)"), in_=PB_d[:])
        TQ = consts.tile([128, NCHUNK, 10], dt.float32)
        nc.sync.dma_start(out=TQ[:].rearrange("p a b -> p (a b)"), in_=TQ_d[:])
        BAS = consts.tile([128, 8 * NPIX], dt.bfloat16)
        nc.sync.dma_start(out=BAS[:], in_=BAS_d[:])

        # Coefficient slabs for ALL m-chunks, n-major so 4 consecutive n
        # flatten to one [128, 128] transpose input.  Rows 24..31 stay zero.
        C = consts.tile([128, NCHUNK, NLOC, KR], dt.float32)
        S = consts.tile([128, 16, NLOC], dt.float32)
        stash = consts.tile([128, NCHUNK, 16, 128], dt.float16)

        def pb(q):
            return PB[:, _QORDER.index(q), :]

        def tq(j, q):
            i = _QORDER.index(q)
            return TQ[:, j, i:i + 1]

        # GPSIMD can't run TensorScalarPtr (per-partition AP scalar), which
        # the t-box ops need, so the coefficient phase runs on DVE.
        g = nc.vector
        nc.gpsimd.memset(C[:, :, :, 24:KR], 0.0)

        def s(i):
            return S[:, i, :]

        # ---- phase 0: coefficient slabs (DVE) ----
        for j in range(NCHUNK):
            def c(r):
                return C[:, j, :, r]

            if j == 0:
                # Engine instructions carry a single HW sync-wait slot, so
                # the first op after the two input DMAs may not wait on both
                # DMA sems at once.  Chain two single-wait ops; the WAW
                # overlap with s(0) orders the real first op after them.
                g.tensor_copy(s(0)[:, 1:2], PB[:, 0, 0:1])
                g.tensor_copy(s(0)[:, 0:1], TQ[:, 0, 0:1])

            g.tensor_scalar(s(0), pb("x0"), tq(j, "x0"), None, op.min)   # xmin
            g.tensor_scalar(s(1), pb("x1"), tq(j, "x1"), None, op.max)   # xmax
            g.tensor_scalar(s(2), pb("y0"), tq(j, "y0"), None, op.min)   # ymin
            g.tensor_scalar(s(3), pb("y1"), tq(j, "y1"), None, op.max)   # ymax
            g.tensor_tensor(s(4), s(1), s(0), op.subtract)               # sx
            g.tensor_tensor(s(5), s(3), s(2), op.subtract)               # sy
            g.tensor_tensor(s(6), s(0), pb("cx"), op.subtract)           # dxp
            g.tensor_tensor(s(7), s(2), pb("cy"), op.subtract)           # dyp
            # a0p = dxp*ctp + dyp*stp ; b0p = dyp*ctp - dxp*stp
            g.tensor_tensor(s(8), s(6), pb("ct"), op.mult)
            g.tensor_tensor(s(9), s(7), pb("st"), op.mult)
            g.tensor_tensor(s(9), s(8), s(9), op.add)                    # a0p
            g.tensor_tensor(s(8), s(7), pb("ct"), op.mult)
            g.tensor_tensor(s(10), s(6), pb("st"), op.mult)
            g.tensor_tensor(s(10), s(8), s(10), op.subtract)             # b0p

            # field A1: P = shw_p - a0p (const row), Q = shw_p + a0p
            g.scalar_tensor_tensor(c(0), s(9), -1.0, pb("shw"), op.mult, op.add)
            g.scalar_tensor_tensor(c(12), s(9), 1.0, pb("shw"), op.mult, op.add)
            # a1p = sx*ctp -> rows 1/13 ; a2p = sy*stp -> rows 2/14
            g.tensor_tensor(s(8), s(4), pb("ct"), op.mult)
            g.tensor_scalar(c(1), s(8), -1.0, None, op.mult)
            g.tensor_copy(c(13), s(8))
            g.tensor_tensor(s(8), s(5), pb("st"), op.mult)
            g.tensor_scalar(c(2), s(8), -1.0, None, op.mult)
            g.tensor_copy(c(14), s(8))
            # field B1 (rows 6-8/18-20; field order is A1,A2,B1,B2)
            g.scalar_tensor_tensor(c(6), s(10), -1.0, pb("shh"), op.mult, op.add)
            g.scalar_tensor_tensor(c(18), s(10), 1.0, pb("shh"), op.mult, op.add)
            # b1p = -sx*stp: P row = +sx*stp, Q row = -sx*stp
            g.tensor_tensor(s(8), s(4), pb("st"), op.mult)
            g.tensor_copy(c(7), s(8))
            g.tensor_scalar(c(19), s(8), -1.0, None, op.mult)
            # b2p = sy*ctp
            g.tensor_tensor(s(8), s(5), pb("ct"), op.mult)
            g.tensor_scalar(c(8), s(8), -1.0, None, op.mult)
            g.tensor_copy(c(20), s(8))
            # target box: dxt/dyt
            g.tensor_scalar(s(12), s(0), tq(j, "cx"), None, op.subtract)
            g.tensor_scalar(s(13), s(2), tq(j, "cy"), None, op.subtract)
            # a0t = dxt*ctt + dyt*stt
            g.tensor_scalar(s(8), s(12), tq(j, "ct"), None, op.mult)
            g.tensor_scalar(s(14), s(13), tq(j, "st"), None, op.mult)
            g.tensor_tensor(s(14), s(8), s(14), op.add)
            # b0t = dyt*ctt - dxt*stt
            g.tensor_scalar(s(8), s(13), tq(j, "ct"), None, op.mult)
            g.tensor_scalar(s(15), s(12), tq(j, "st"), None, op.mult)
            g.tensor_tensor(s(15), s(8), s(15), op.subtract)
            # field A2 const rows (rows 3-5/15-17)
            g.tensor_scalar(c(3), s(14), -1.0, tq(j, "shw"), op.mult, op.add)
            g.tensor_scalar(c(15), s(14), 1.0, tq(j, "shw"), op.mult, op.add)
            # a1t = sx*ctt ; a2t = sy*stt
            g.tensor_scalar(s(8), s(4), tq(j, "ct"), None, op.mult)
            g.tensor_scalar(c(4), s(8), -1.0, None, op.mult)
            g.tensor_copy(c(16), s(8))
            g.tensor_scalar(s(8), s(5), tq(j, "st"), None, op.mult)
            g.tensor_scalar(c(5), s(8), -1.0, None, op.mult)
            g.tensor_copy(c(17), s(8))
            # field B2 const rows
            g.tensor_scalar(c(9), s(15), -1.0, tq(j, "shh"), op.mult, op.add)
            g.tensor_scalar(c(21), s(15), 1.0, tq(j, "shh"), op.mult, op.add)
            # b1t = -sx*stt ; b2t = sy*ctt
            g.tensor_scalar(s(8), s(4), tq(j, "st"), None, op.mult)
            g.tensor_copy(c(10), s(8))
            g.tensor_scalar(c(22), s(8), -1.0, None, op.mult)
            g.tensor_scalar(s(8), s(5), tq(j, "ct"), None, op.mult)
            g.tensor_scalar(c(11), s(8), -1.0, None, op.mult)
            g.tensor_copy(c(23), s(8))

        # ---- phase 1+2 interleaved per m-chunk: transposes flow with MMs ----
        first = True
        for j in range(NCHUNK):
            Ssum = accp.tile([128, NLOC], dt.float32, tag="Ssum")
            Isum = accp.tile([128, NLOC], dt.float32, tag="Isum")

            def transpose_group(gg):
                # stash copy on VE, not ACT: an ACT copy in the scalar
                # engine's in-order queue delays the next sigmoid pair by
                # ~1.6us every group (measured).
                T = psum.tile([128, 8 * NPIX], dt.float32, tag="pq")
                nc.tensor.transpose(
                    T[:, 0:128],
                    C[:, j, 4 * gg:4 * gg + 4, :].rearrange("p n r -> p (n r)"),
                    ident[:])
                nc.vector.tensor_copy(stash[:, j, gg, :], T[:, 0:128])

            if first:
                # Warm the PE clock on the BAS DMA sem (single-wait LDW)
                # before the first real transpose, which must wait on
                # the DVE-written C.
                Tw = psum.tile([128, 8 * NPIX], dt.float32, tag="pq")
                nc.tensor.matmul(
                    Tw[:, 0:512], BAS[0:32, 0:128], BAS[0:32, 0:512],
                    start=True, stop=True)
                first = False
            transpose_group(0)

            for grp in range(NLOC // 4):
                # Software-pipeline the NEXT group's transpose+copy ahead of
                # this group's heavy VE ops, so its stash copy sits early in
                # the VE queue and the PE never stalls on a late copy.
                if grp + 1 < NLOC // 4:
                    transpose_group(grp + 1)

                # Process the whole group (4 n) per VE instruction set via
                # [128, 4, X] views (inner dim dense, so the DVE 2x bf16
                # mode still applies), amortizing per-instruction overhead.
                n0 = 4 * grp
                sig = sigp.tile([128, 4, 8 * NPIX], dt.bfloat16, tag="sig")
                for k in range(4):
                    PQ = psum.tile([128, 8 * NPIX], dt.float32, tag="pq")
                    for h in range(4):
                        nc.tensor.matmul(
                            PQ[:, h * 512:(h + 1) * 512],
                            stash[32 * k:32 * (k + 1), j, grp, :],
                            BAS[32 * k:32 * (k + 1), h * 512:(h + 1) * 512],
                            start=True, stop=True,
                            tile_position=(32 * k, 0))
                    nc.scalar.activation(
                        sig[:, k, :], PQ[:], AF.Sigmoid, 0.0, K)
                Gm = vminp.tile([128, 4, 4 * NPIX], dt.bfloat16, tag="Gm")
                nc.vector.tensor_tensor(
                    Gm[:], sig[:, :, 0:1024], sig[:, :, 1024:2048], op.mult)
                Fp = fpp.tile([128, 4, 2 * NPIX], dt.bfloat16, tag="Fp")
                F12 = fpp.tile([128, 4, NPIX], dt.bfloat16, tag="F12")
                nc.vector.tensor_mul(
                    Fp[:], Gm[:, :, 0:512], Gm[:, :, 512:1024])
                nc.vector.tensor_reduce(
                    Ssum[:, n0:n0 + 4], Fp[:], mybir.AxisListType.X, op.add)
                nc.vector.tensor_mul(
                    F12[:], Fp[:, :, 0:NPIX], Fp[:, :, NPIX:2 * NPIX])
                nc.vector.tensor_reduce(
                    Isum[:, n0:n0 + 4], F12[:], mybir.AxisListType.X, op.add)

            # ---- epilogue: piou = inter / (stot - inter + eps) ----
            union = accp.tile([128, NLOC], dt.float32, tag="union")
            nc.vector.scalar_tensor_tensor(
                union[:], Isum[:], -1.0, Ssum[:], op.mult, op.add)
            nc.vector.tensor_scalar(union[:], union[:], float(EPS), None, op.add)
            rec = accp.tile([128, NLOC], dt.float32, tag="rec")
            nc.vector.reciprocal(rec[:], union[:])
            piou = accp.tile([128, NLOC], dt.float32, tag="piou")
            nc.vector.tensor_tensor(piou[:], Isum[:], rec[:], op.mult)
            nc.sync.dma_start(out=OUT_d[j * 128:(j + 1) * 128, :], in_=piou[:])

    nc.finalize()
    return nc


def _get_compiled():
    if "nc" not in _cache:
        _cache["nc"] = _build_nc()
    return _cache["nc"]


def _get_runner():
    """Persistent jitted shard_map callable (mirrors bass2jax.run_bass_via_pjrt
    but caches the traced/jitted function so steady-state calls skip jax
    re-trace + re-lowering, which dominate the per-call wall time)."""
    if "runner" in _cache:
        return _cache["runner"]

    import jax
    import numpy as _np
    from jax.experimental.shard_map import shard_map
    from jax.sharding import Mesh, PartitionSpec

    import concourse.bass2jax as b2j
    from concourse import mybir

    nc = _get_compiled()
    b2j.install_neuronx_cc_hook()
    partition_name = nc.partition_id_tensor.name if nc.partition_id_tensor else None

    in_names, out_names, out_avals, zero_shapes = [], [], [], []
    for alloc in nc.m.functions[0].allocations:
        if not isinstance(alloc, mybir.MemoryLocationSet):
            continue
        name = alloc.memorylocations[0].name
        if alloc.kind == "ExternalInput":
            if name != partition_name:
                in_names.append(name)
        elif alloc.kind == "ExternalOutput":
            out_names.append(name)
            shape = tuple(alloc.tensor_shape)
            dtype = mybir.dt.np(alloc.dtype)
            out_avals.append(jax.core.ShapedArray(shape, dtype))
            zero_shapes.append((shape, dtype))
    n_params = len(in_names)
    n_outs = len(out_avals)
    all_names = list(in_names) + list(out_names)
    if partition_name is not None:
        all_names.append(partition_name)
    donate = tuple(range(n_params, n_params + n_outs))

    def _body(*args):
        operands = list(args)
        if partition_name is not None:
            operands.append(b2j.partition_id_tensor())
        outs = b2j._bass_exec_p.bind(
            *operands,
            out_avals=tuple(out_avals),
            in_names=tuple(all_names),
            out_names=tuple(out_names),
            lowering_input_output_aliases=(),
            sim_require_finite=True,
            sim_require_nnan=True,
            nc=nc,
        )
        return tuple(outs)

    devices = jax.devices()[:NC]
    assert len(devices) >= NC, f"need {NC} devices, have {len(jax.devices())}"
    mesh = Mesh(_np.asarray(devices), ("core",))
    in_specs = (PartitionSpec("core"),) * (n_params + n_outs)
    out_specs = (PartitionSpec("core"),) * n_outs
    sharded = jax.jit(
        shard_map(_body, mesh=mesh, in_specs=in_specs, out_specs=out_specs,
                  check_rep=False),
        donate_argnums=donate,
        keep_unused=True,
    )

    def run(in_maps):
        concat_in = [
            np.concatenate([np.asarray(in_maps[c][nm]) for c in range(NC)], axis=0)
            for nm in in_names
        ]
        zeros = [np.zeros((NC * sh[0], *sh[1:]), dtp) for sh, dtp in zero_shapes]
        out_arrs = sharded(*concat_in, *zeros)
        return [
            {nm: np.asarray(out_arrs[i]).reshape(NC, *out_avals[i].shape)[c]
             for i, nm in enumerate(out_names)}
            for c in range(NC)
        ]

    _cache["runner"] = run
    return run


def kernel(loc_p, loc_t, grid):
    assert int(grid) == G
    loc_p = np.asarray(loc_p, dtype=np.float32)
    loc_t = np.asarray(loc_t, dtype=np.float32)
    basis, TQ, PBs = _host_constants(loc_p, loc_t)
    in_maps = [{"PB": PBs[c], "TQ": TQ, "BASIS": basis} for c in range(NC)]

    try:
        res = _get_runner()(in_maps)
    except Exception:
        # Robust fallback: the stock (slower) dispatch path.
        from concourse.bass_utils import run_bass_kernel_spmd

        res = run_bass_kernel_spmd(
            _get_compiled(), in_maps, core_ids=list(range(NC))).results

    out = np.empty((N, M), dtype=np.float32)
    for c in range(NC):
        out[c * NLOC:(c + 1) * NLOC, :] = res[c]["OUT"].T
    return out



# revision 3
# speedup vs baseline: 8.3101x; 8.3101x over previous
"""Sparse PIoU (pixel-wise IoU) pairwise matrix kernel for Trainium2, 8 cores.

Math: for each pair (pred box n, target box m) the reference samples a 16x16
grid of the joint AABB and evaluates soft memberships
F = sigmoid(k(w/2-|A|)) * sigmoid(k(h/2-|B|)) per box, with (A, B) the pixel
offsets rotated into the box frame.  A and B are affine in the grid coords
(ug, uh), so the sigmoid args (s/2 -+ A) for all 256 pixels x 4 fields x
{P,Q} come from one K=24 matmul against a constant basis; since
P + Q = s >= 8 and k = 10, sigmoid(kP)*sigmoid(kQ) == sigmoid(k*min(P,Q)) to
machine precision, giving the |.| for free.

Sparsity: boxes are small (8..96 px) in a 640x640 field, so only ~8% of the
512x512 pairs have overlapping (2px-dilated) AABBs; every excluded pair has
true PIoU < 1e-14 (sigmoid tails at >= 2px separation), so the host computes
the dilated-AABB mask, packs ONLY surviving pairs one-per-partition into
units of 128, and scatters device results into a zero matrix.  ~21.3k pairs
-> 21 units/core instead of the dense 256 (n x m-chunk) units: ~12x less
device work, identical numerics on every surviving pair.

Device pipeline per unit u (128 pairs):
    PE  : 4 matmuls [24,128]x[24,512] -> PSUM [128, 2048] = P|Q sig args
    ACT : sig = Sigmoid(K * PQ)            [128, 2048] bf16 (one instruction)
    DVE : Gm  = sigP * sigQ                [128, 1024] (field memberships)
    DVE : Fp  = gA * gB,  accum -> S[u]    (fused product+reduce, STT)
    DVE : F12 = F1 * F2,  accum -> I[u]    (fused product+reduce, STT)
Epilogue (once): piou = I / (S - I + eps)  [128, U] -> one DMA out.

Host precomputes the per-pair coefficient slab directly in the transposed
fp16 layout the PE wants (lhsT [24, pairs]), eliminating the on-device
coefficient phase, PE transposes and stash copies of the dense kernel.

Dispatch uses a persistent jitted shard_map callable (cached per unit-count
U) so steady-state calls skip jax re-trace/re-lowering.
"""

import numpy as np

N = 512
M = 512
G = 16
NPIX = G * G
K_SLOPE = np.float32(10.0)
EPS = np.float32(1e-6)
NC = 8
KR = 32   # coefficient rows (24 used + 8 zero pad)
DELTA = np.float32(2.0)  # AABB dilation margin in px (excluded-pair PIoU < 1e-14)

_cache = {}


def _derived(b):
    # b: [K,5] float32 -> per-box derived quantities (all float32)
    cx, cy, w, h, t = (b[:, i].astype(np.float32) for i in range(5))
    c, s = np.cos(t).astype(np.float32), np.sin(t).astype(np.float32)
    hw = np.float32(0.5) * (w * np.abs(c) + h * np.abs(s))
    hh = np.float32(0.5) * (w * np.abs(s) + h * np.abs(c))
    return dict(
        cx=cx, cy=cy, ct=c, st=s,
        shw=np.float32(0.5) * w, shh=np.float32(0.5) * h,
        x0=cx - hw, x1=cx + hw, y0=cy - hh, y1=cy + hh,
    )


def _basis():
    # [KR, 2048] bf16 (values exact): P cols 0..1023, Q cols 1024..2047; field
    # f at cols f*256..(f+1)*256 uses rows 3f..3f+2 (P) / 12+3f.. (Q)
    # = (1, Ug, Uh).  Pixel p = h*G+g -> Ug[p]=u[g], Uh[p]=u[h].
    u = (np.arange(G, dtype=np.float32) + np.float32(0.5)) / np.float32(G)
    Ug = np.tile(u, G)
    Uh = np.repeat(u, G)
    bas = np.zeros((KR, 8 * NPIX), dtype=np.float32)
    for f in range(4):
        for blk, r0 in ((0, 0), (1, 12)):
            c0 = blk * 4 * NPIX + f * NPIX
            bas[r0 + 3 * f + 0, c0:c0 + NPIX] = 1.0
            bas[r0 + 3 * f + 1, c0:c0 + NPIX] = Ug
            bas[r0 + 3 * f + 2, c0:c0 + NPIX] = Uh
    import ml_dtypes

    return bas.astype(ml_dtypes.bfloat16)


def _pair_coeffs(P, T, n_idx, m_idx):
    """[24, npairs] float32 coefficient slab for the given (n, m) pairs.

    Row 3f+r holds the P-arg coeff of field f on basis fn r in (1, ug, uh);
    row 12+3f+r the Q-arg coeff.  Field order: A-pred, A-targ, B-pred, B-targ
    so that Gm pairs A1|A2 with B1|B2 and Fp pairs F1 with F2 downstream.
    """
    p = {k: v[n_idx] for k, v in P.items()}
    t = {k: v[m_idx] for k, v in T.items()}
    xmin = np.minimum(p["x0"], t["x0"])
    xmax = np.maximum(p["x1"], t["x1"])
    ymin = np.minimum(p["y0"], t["y0"])
    ymax = np.maximum(p["y1"], t["y1"])
    sx = xmax - xmin
    sy = ymax - ymin
    C = np.empty((24, n_idx.size), dtype=np.float32)
    for f, (b, ab) in enumerate(((p, "a"), (t, "a"), (p, "b"), (t, "b"))):
        dx0 = xmin - b["cx"]
        dy0 = ymin - b["cy"]
        if ab == "a":
            c0 = dx0 * b["ct"] + dy0 * b["st"]
            c1 = sx * b["ct"]
            c2 = sy * b["st"]
            half = b["shw"]
        else:
            c0 = dy0 * b["ct"] - dx0 * b["st"]
            c1 = -sx * b["st"]
            c2 = sy * b["ct"]
            half = b["shh"]
        C[3 * f + 0] = half - c0
        C[3 * f + 1] = -c1
        C[3 * f + 2] = -c2
        C[12 + 3 * f + 0] = half + c0
        C[12 + 3 * f + 1] = c1
        C[12 + 3 * f + 2] = c2
    return C


def _build_nc(U):
    from contextlib import ExitStack

    import concourse.bacc as bacc
    import concourse.tile as tile
    from concourse import mybir

    dt = mybir.dt
    op = mybir.AluOpType
    AF = mybir.ActivationFunctionType
    K = float(K_SLOPE)

    nc = bacc.Bacc(None, target_bir_lowering=False)
    LH_d = nc.declare_dram_parameter("LHST", [KR, U * 128], dt.float16, isOutput=False)
    BAS_d = nc.declare_dram_parameter("BASIS", [KR, 8 * NPIX], dt.bfloat16, isOutput=False)
    OUT_d = nc.declare_dram_parameter("OUT", [128, U], dt.float32, isOutput=True)

    with tile.TileContext(nc) as tc, ExitStack() as ctx:
        consts = ctx.enter_context(tc.tile_pool(name="consts", bufs=1))
        sigp = ctx.enter_context(tc.tile_pool(name="sigp", bufs=4))
        gmp = ctx.enter_context(tc.tile_pool(name="gmp", bufs=3))
        fpp = ctx.enter_context(tc.tile_pool(name="fpp", bufs=3))
        accp = ctx.enter_context(tc.tile_pool(name="accp", bufs=1))
        psum = ctx.enter_context(tc.tile_pool(name="psum", bufs=2, space="PSUM"))

        BAS = consts.tile([KR, 8 * NPIX], dt.bfloat16)
        nc.sync.dma_start(out=BAS[:], in_=BAS_d[:])
        # Coefficient slabs in 4 chunks so unit 0 can start before the tail
        # of the (~170 KB) slab DMA lands.
        LH = consts.tile([KR, U, 128], dt.float16)
        nchunk = min(4, U)
        bounds = [U * i // nchunk for i in range(nchunk + 1)]
        for i in range(nchunk):
            lo, hi = bounds[i], bounds[i + 1]
            nc.sync.dma_start(
                out=LH[:, lo:hi, :].rearrange("p a b -> p (a b)"),
                in_=LH_d[:, lo * 128:hi * 128])

        Ssum = accp.tile([128, U], dt.float32)
        Isum = accp.tile([128, U], dt.float32)

        # Warm the PE clock on the BAS DMA sem before the first real matmul.
        Tw = psum.tile([128, 8 * NPIX], dt.float32, tag="pq")
        nc.tensor.matmul(
            Tw[:, 0:512], BAS[0:KR, 0:128], BAS[0:KR, 0:512],
            start=True, stop=True)

        for u in range(U):
            PQ = psum.tile([128, 8 * NPIX], dt.float32, tag="pq")
            for h in range(4):
                nc.tensor.matmul(
                    PQ[:, h * 512:(h + 1) * 512],
                    LH[:, u, :],
                    BAS[:, h * 512:(h + 1) * 512],
                    start=True, stop=True)
            sig = sigp.tile([128, 8 * NPIX], dt.bfloat16, tag="sig")
            nc.scalar.activation(sig[:], PQ[:], AF.Sigmoid, 0.0, K)
            Gm = gmp.tile([128, 4 * NPIX], dt.bfloat16, tag="Gm")
            nc.vector.tensor_tensor(
                Gm[:], sig[:, 0:1024], sig[:, 1024:2048], op.mult)
            Fp = fpp.tile([128, 2 * NPIX], dt.bfloat16, tag="Fp")
            nc.vector.scalar_tensor_tensor(
                Fp[:], Gm[:, 0:512], 1.0, Gm[:, 512:1024], op.mult, op.mult,
                accum_out=Ssum[:, u:u + 1])
            F12 = fpp.tile([128, NPIX], dt.bfloat16, tag="F12")
            nc.vector.scalar_tensor_tensor(
                F12[:], Fp[:, 0:NPIX], 1.0, Fp[:, NPIX:2 * NPIX], op.mult, op.mult,
                accum_out=Isum[:, u:u + 1])

        # ---- epilogue: piou = I / (S - I + eps) ----
        union = accp.tile([128, U], dt.float32)
        nc.vector.scalar_tensor_tensor(
            union[:], Ssum[:], float(EPS), Isum[:], op.add, op.subtract)
        rec = accp.tile([128, U], dt.float32)
        nc.vector.reciprocal(rec[:], union[:])
        piou = accp.tile([128, U], dt.float32)
        nc.vector.tensor_tensor(piou[:], Isum[:], rec[:], op.mult)
        nc.sync.dma_start(out=OUT_d[:], in_=piou[:])

    nc.finalize()
    return nc


def _get_compiled(U):
    key = ("nc", U)
    if key not in _cache:
        _cache[key] = _build_nc(U)
    return _cache[key]


def _get_runner(U):
    """Persistent jitted shard_map callable (cached per unit count U)."""
    key = ("runner", U)
    if key in _cache:
        return _cache[key]

    import jax
    import numpy as _np
    from jax.experimental.shard_map import shard_map
    from jax.sharding import Mesh, PartitionSpec

    import concourse.bass2jax as b2j
    from concourse import mybir

    nc = _get_compiled(U)
    b2j.install_neuronx_cc_hook()
    partition_name = nc.partition_id_tensor.name if nc.partition_id_tensor else None

    in_names, out_names, out_avals, zero_shapes = [], [], [], []
    for alloc in nc.m.functions[0].allocations:
        if not isinstance(alloc, mybir.MemoryLocationSet):
            continue
        name = alloc.memorylocations[0].name
        if alloc.kind == "ExternalInput":
            if name != partition_name:
                in_names.append(name)
        elif alloc.kind == "ExternalOutput":
            out_names.append(name)
            shape = tuple(alloc.tensor_shape)
            dtype = mybir.dt.np(alloc.dtype)
            out_avals.append(jax.core.ShapedArray(shape, dtype))
            zero_shapes.append((shape, dtype))
    n_params = len(in_names)
    n_outs = len(out_avals)
    all_names = list(in_names) + list(out_names)
    if partition_name is not None:
        all_names.append(partition_name)
    donate = tuple(range(n_params, n_params + n_outs))

    def _body(*args):
        operands = list(args)
        if partition_name is not None:
            operands.append(b2j.partition_id_tensor())
        outs = b2j._bass_exec_p.bind(
            *operands,
            out_avals=tuple(out_avals),
            in_names=tuple(all_names),
            out_names=tuple(out_names),
            lowering_input_output_aliases=(),
            sim_require_finite=True,
            sim_require_nnan=True,
            nc=nc,
        )
        return tuple(outs)

    devices = jax.devices()[:NC]
    assert len(devices) >= NC, f"need {NC} devices, have {len(jax.devices())}"
    mesh = Mesh(_np.asarray(devices), ("core",))
    in_specs = (PartitionSpec("core"),) * (n_params + n_outs)
    out_specs = (PartitionSpec("core"),) * n_outs
    sharded = jax.jit(
        shard_map(_body, mesh=mesh, in_specs=in_specs, out_specs=out_specs,
                  check_rep=False),
        donate_argnums=donate,
        keep_unused=True,
    )

    def run(in_maps):
        concat_in = [
            np.concatenate([np.asarray(in_maps[c][nm]) for c in range(NC)], axis=0)
            for nm in in_names
        ]
        zeros = [np.zeros((NC * sh[0], *sh[1:]), dtp) for sh, dtp in zero_shapes]
        out_arrs = sharded(*concat_in, *zeros)
        return [
            {nm: np.asarray(out_arrs[i]).reshape(NC, *out_avals[i].shape)[c]
             for i, nm in enumerate(out_names)}
            for c in range(NC)
        ]

    _cache[key] = run
    return run


def kernel(loc_p, loc_t, grid):
    assert int(grid) == G
    loc_p = np.asarray(loc_p, dtype=np.float32)
    loc_t = np.asarray(loc_t, dtype=np.float32)
    n_p, n_t = loc_p.shape[0], loc_t.shape[0]

    P = _derived(loc_p)
    T = _derived(loc_t)

    # Pairs whose DELTA-dilated AABBs overlap; everything else is < 1e-14.
    ox = (P["x0"][:, None] <= T["x1"][None, :] + DELTA) & \
         (T["x0"][None, :] <= P["x1"][:, None] + DELTA)
    oy = (P["y0"][:, None] <= T["y1"][None, :] + DELTA) & \
         (T["y0"][None, :] <= P["y1"][:, None] + DELTA)
    idx = np.argwhere(ox & oy)

    # Round-robin pairs over cores; pad each core to U*128 with dummy pairs.
    per_core = [idx[c::NC] for c in range(NC)]
    U = max(1, -(-max(len(pc) for pc in per_core) // 128))

    basis = _basis()
    in_maps = []
    for c in range(NC):
        pc = per_core[c]
        lh = np.zeros((KR, U * 128), dtype=np.float32)
        if len(pc):
            lh[:24, :len(pc)] = _pair_coeffs(P, T, pc[:, 0], pc[:, 1])
        in_maps.append({"LHST": lh.astype(np.float16),
                        "BASIS": basis})

    try:
        res = _get_runner(U)(in_maps)
    except Exception:
        # Robust fallback: the stock (slower) dispatch path.
        from concourse.bass_utils import run_bass_kernel_spmd

        res = run_bass_kernel_spmd(
            _get_compiled(U), in_maps, core_ids=list(range(NC))).results

    out = np.zeros((n_p, n_t), dtype=np.float32)
    for c in range(NC):
        pc = per_core[c]
        if len(pc):
            vals = res[c]["OUT"].T.reshape(-1)[:len(pc)]
            out[pc[:, 0], pc[:, 1]] = vals
    return out


# revision 8
# speedup vs baseline: 9.0813x; 1.0928x over previous
"""Sparse PIoU (pixel-wise IoU) pairwise matrix kernel for Trainium2, 8 cores.

Math: for each pair (pred box n, target box m) the reference samples a 16x16
grid of the joint AABB and evaluates soft memberships
F = sigmoid(k(w/2-|A|)) * sigmoid(k(h/2-|B|)) per box, with (A, B) the pixel
offsets rotated into the box frame.  A and B are affine in the grid coords
(ug, uh), so the sigmoid args (s/2 -+ A) for all 256 pixels x 4 fields x
{P,Q} come from one K=24 matmul against a constant basis; since
P + Q = s >= 8 and k = 10, sigmoid(kP)*sigmoid(kQ) == sigmoid(k*min(P,Q)) to
machine precision, giving the |.| for free.

Sparsity: boxes are small (8..96 px) in a 640x640 field, so only ~8% of the
512x512 pairs have overlapping (2px-dilated) AABBs; every excluded pair has
true PIoU < 1e-14 (sigmoid tails at >= 2px separation), so the host computes
the dilated-AABB mask, packs ONLY surviving pairs one-per-partition into
units of 128, and scatters device results into a zero matrix.  ~21.3k pairs
-> 21 units/core instead of the dense 256 (n x m-chunk) units: ~12x less
device work, identical numerics on every surviving pair.

Device pipeline per unit u (128 pairs):
    PE  : 4 matmuls [24,128]x[24,512] -> PSUM [128, 2048] = P|Q sig args
    ACT : sig = Sigmoid(K * PQ)            [128, 2048] bf16 (one instruction)
    DVE : Gm  = sigP * sigQ                [128, 1024] (field memberships)
    DVE : Fp  = gA * gB,  accum -> S[u]    (fused product+reduce, STT)
    DVE : F12 = F1 * F2,  accum -> I[u]    (fused product+reduce, STT)
Epilogue (once): piou = I / (S - I + eps)  [128, U] -> one DMA out.

Host precomputes the per-pair coefficient slab directly in the transposed
fp16 layout the PE wants (lhsT [24, pairs]), eliminating the on-device
coefficient phase, PE transposes and stash copies of the dense kernel.

Dispatch uses a persistent jitted shard_map callable (cached per unit-count
U) so steady-state calls skip jax re-trace/re-lowering.
"""

import numpy as np

N = 512
M = 512
G = 16
NPIX = G * G
K_SLOPE = np.float32(10.0)
EPS = np.float32(1e-6)
NC = 8
KR = 32   # coefficient rows (24 used + 8 zero pad)
DELTA = np.float32(2.0)  # AABB dilation margin in px (excluded-pair PIoU < 1e-14)

_cache = {}


def _derived(b):
    # b: [K,5] float32 -> per-box derived quantities (all float32)
    cx, cy, w, h, t = (b[:, i].astype(np.float32) for i in range(5))
    c, s = np.cos(t).astype(np.float32), np.sin(t).astype(np.float32)
    hw = np.float32(0.5) * (w * np.abs(c) + h * np.abs(s))
    hh = np.float32(0.5) * (w * np.abs(s) + h * np.abs(c))
    return dict(
        cx=cx, cy=cy, ct=c, st=s,
        shw=np.float32(0.5) * w, shh=np.float32(0.5) * h,
        x0=cx - hw, x1=cx + hw, y0=cy - hh, y1=cy + hh,
    )


def _basis():
    # [KR, 2048] bf16 (values exact): P cols 0..1023, Q cols 1024..2047; field
    # f at cols f*256..(f+1)*256 uses rows 3f..3f+2 (P) / 12+3f.. (Q)
    # = (1, Ug, Uh).  Pixel p = h*G+g -> Ug[p]=u[g], Uh[p]=u[h].
    u = (np.arange(G, dtype=np.float32) + np.float32(0.5)) / np.float32(G)
    Ug = np.tile(u, G)
    Uh = np.repeat(u, G)
    bas = np.zeros((KR, 8 * NPIX), dtype=np.float32)
    for f in range(4):
        for blk, r0 in ((0, 0), (1, 12)):
            c0 = blk * 4 * NPIX + f * NPIX
            bas[r0 + 3 * f + 0, c0:c0 + NPIX] = 1.0
            bas[r0 + 3 * f + 1, c0:c0 + NPIX] = Ug
            bas[r0 + 3 * f + 2, c0:c0 + NPIX] = Uh
    import ml_dtypes

    return bas.astype(ml_dtypes.bfloat16)


def _sat_separated(P, T, n_idx, m_idx, margin):
    """True for pairs whose margin-dilated rotated boxes are disjoint
    (separating-axis test on the 4 edge normals).  At k=10, a separation
    margin of 2px bounds the true PIoU of excluded pairs below ~1e-12."""
    dcx = T["cx"][m_idx] - P["cx"][n_idx]
    dcy = T["cy"][m_idx] - P["cy"][n_idx]
    sep = np.zeros(n_idx.size, dtype=bool)
    for src in (0, 1):
        B1, i1 = (P, n_idx) if src == 0 else (T, m_idx)
        B2, i2 = (T, m_idx) if src == 0 else (P, n_idx)
        ct, st = B1["ct"][i1], B1["st"][i1]
        c2, s2 = B2["ct"][i2], B2["st"][i2]
        for ax in range(2):
            ux, uy = (ct, st) if ax == 0 else (-st, ct)
            e1 = B1["shw" if ax == 0 else "shh"][i1]
            e2 = (B2["shw"][i2] * np.abs(ux * c2 + uy * s2)
                  + B2["shh"][i2] * np.abs(-ux * s2 + uy * c2))
            sep |= np.abs(ux * dcx + uy * dcy) > e1 + e2 + margin
    return sep


def _pair_coeffs(P, T, n_idx, m_idx):
    """[24, npairs] float32 coefficient slab for the given (n, m) pairs.

    Row 3f+r holds the P-arg coeff of field f on basis fn r in (1, ug, uh);
    row 12+3f+r the Q-arg coeff.  Field order: A-pred, A-targ, B-pred, B-targ
    so that Gm pairs A1|A2 with B1|B2 and Fp pairs F1 with F2 downstream.
    """
    p = {k: v[n_idx] for k, v in P.items()}
    t = {k: v[m_idx] for k, v in T.items()}
    xmin = np.minimum(p["x0"], t["x0"])
    xmax = np.maximum(p["x1"], t["x1"])
    ymin = np.minimum(p["y0"], t["y0"])
    ymax = np.maximum(p["y1"], t["y1"])
    sx = xmax - xmin
    sy = ymax - ymin
    C = np.empty((24, n_idx.size), dtype=np.float32)
    for f, (b, ab) in enumerate(((p, "a"), (t, "a"), (p, "b"), (t, "b"))):
        dx0 = xmin - b["cx"]
        dy0 = ymin - b["cy"]
        if ab == "a":
            c0 = dx0 * b["ct"] + dy0 * b["st"]
            c1 = sx * b["ct"]
            c2 = sy * b["st"]
            half = b["shw"]
        else:
            c0 = dy0 * b["ct"] - dx0 * b["st"]
            c1 = -sx * b["st"]
            c2 = sy * b["ct"]
            half = b["shh"]
        C[3 * f + 0] = half - c0
        C[3 * f + 1] = -c1
        C[3 * f + 2] = -c2
        C[12 + 3 * f + 0] = half + c0
        C[12 + 3 * f + 1] = c1
        C[12 + 3 * f + 2] = c2
    return C


def _build_nc(U):
    from contextlib import ExitStack

    import concourse.bacc as bacc
    import concourse.tile as tile
    from concourse import mybir

    dt = mybir.dt
    op = mybir.AluOpType
    AF = mybir.ActivationFunctionType
    K = float(K_SLOPE)

    nc = bacc.Bacc(None, target_bir_lowering=False)
    LH_d = nc.declare_dram_parameter("LHST", [KR, U * 128], dt.float16, isOutput=False)
    BAS_d = nc.declare_dram_parameter("BASIS", [KR, 8 * NPIX], dt.bfloat16, isOutput=False)
    OUT_d = nc.declare_dram_parameter("OUT", [128, U], dt.float32, isOutput=True)

    with tile.TileContext(nc) as tc, ExitStack() as ctx:
        consts = ctx.enter_context(tc.tile_pool(name="consts", bufs=1))
        sigp = ctx.enter_context(tc.tile_pool(name="sigp", bufs=4))
        gmp = ctx.enter_context(tc.tile_pool(name="gmp", bufs=3))
        fpp = ctx.enter_context(tc.tile_pool(name="fpp", bufs=3))
        accp = ctx.enter_context(tc.tile_pool(name="accp", bufs=1))
        psum = ctx.enter_context(tc.tile_pool(name="psum", bufs=2, space="PSUM"))

        # Input DMAs fan out over engine queues so they trigger concurrently
        # right after the post-preamble barrier instead of serializing on the
        # sync queue (~600ns per trigger).
        BAS = consts.tile([KR, 8 * NPIX], dt.bfloat16)
        nc.scalar.dma_start(out=BAS[:], in_=BAS_d[:])
        LH = consts.tile([KR, U, 128], dt.float16)
        nchunk = min(4, U)
        bounds = [U * i // nchunk for i in range(nchunk + 1)]
        qs = [nc.sync, nc.gpsimd]
        for i in range(nchunk):
            lo, hi = bounds[i], bounds[i + 1]
            qs[i % len(qs)].dma_start(
                out=LH[:, lo:hi, :].rearrange("p a b -> p (a b)"),
                in_=LH_d[:, lo * 128:hi * 128])

        Ssum = accp.tile([128, U], dt.float32)
        Isum = accp.tile([128, U], dt.float32)

        # Warm the PE clock on the BAS DMA sem before the first real matmul.
        Tw = psum.tile([128, 8 * NPIX], dt.float32, tag="pq")
        nc.tensor.matmul(
            Tw[:, 0:512], BAS[0:KR, 0:128], BAS[0:KR, 0:512],
            start=True, stop=True)

        # Epilogue (piou = I / (S - I + eps)) runs in two chunks: the first
        # U1 units finalize + DMA out while the ACT stream is still running,
        # shortening the post-stream tail to the last few units only.
        U1 = max(0, U - 4)

        def epilogue(lo, hi):
            w = hi - lo
            union = accp.tile([128, w], dt.float32, tag=f"un{lo}")
            nc.vector.scalar_tensor_tensor(
                union[:], Ssum[:, lo:hi], float(EPS), Isum[:, lo:hi],
                op.add, op.subtract)
            rec = accp.tile([128, w], dt.float32, tag=f"rec{lo}")
            nc.vector.reciprocal(rec[:], union[:])
            piou = accp.tile([128, w], dt.float32, tag=f"pio{lo}")
            nc.vector.tensor_tensor(piou[:], Isum[:, lo:hi], rec[:], op.mult)
            nc.sync.dma_start(out=OUT_d[:, lo:hi], in_=piou[:])

        for u in range(U):
            PQ = psum.tile([128, 8 * NPIX], dt.float32, tag="pq")
            for h in range(4):
                nc.tensor.matmul(
                    PQ[:, h * 512:(h + 1) * 512],
                    LH[:, u, :],
                    BAS[:, h * 512:(h + 1) * 512],
                    start=True, stop=True)
            sig = sigp.tile([128, 8 * NPIX], dt.bfloat16, tag="sig")
            nc.scalar.activation(sig[:], PQ[:], AF.Sigmoid, 0.0, K)
            Gm = gmp.tile([128, 4 * NPIX], dt.bfloat16, tag="Gm")
            nc.vector.tensor_tensor(
                Gm[:], sig[:, 0:1024], sig[:, 1024:2048], op.mult)
            Fp = fpp.tile([128, 2 * NPIX], dt.bfloat16, tag="Fp")
            nc.vector.scalar_tensor_tensor(
                Fp[:], Gm[:, 0:512], 1.0, Gm[:, 512:1024], op.mult, op.mult,
                accum_out=Ssum[:, u:u + 1])
            F12 = fpp.tile([128, NPIX], dt.bfloat16, tag="F12")
            nc.vector.scalar_tensor_tensor(
                F12[:], Fp[:, 0:NPIX], 1.0, Fp[:, NPIX:2 * NPIX], op.mult, op.mult,
                accum_out=Isum[:, u:u + 1])
            if u == U1 - 1 and U1 < U:
                epilogue(0, U1)

        epilogue(U1, U) if U1 < U else epilogue(0, U)

    nc.finalize()
    return nc


def _get_compiled(U):
    key = ("nc", U)
    if key not in _cache:
        _cache[key] = _build_nc(U)
    return _cache[key]


def _get_runner(U):
    """Persistent jitted shard_map callable (cached per unit count U)."""
    key = ("runner", U)
    if key in _cache:
        return _cache[key]

    import jax
    import numpy as _np
    from jax.experimental.shard_map import shard_map
    from jax.sharding import Mesh, PartitionSpec

    import concourse.bass2jax as b2j
    from concourse import mybir

    nc = _get_compiled(U)
    b2j.install_neuronx_cc_hook()
    partition_name = nc.partition_id_tensor.name if nc.partition_id_tensor else None

    in_names, out_names, out_avals, zero_shapes = [], [], [], []
    for alloc in nc.m.functions[0].allocations:
        if not isinstance(alloc, mybir.MemoryLocationSet):
            continue
        name = alloc.memorylocations[0].name
        if alloc.kind == "ExternalInput":
            if name != partition_name:
                in_names.append(name)
        elif alloc.kind == "ExternalOutput":
            out_names.append(name)
            shape = tuple(alloc.tensor_shape)
            dtype = mybir.dt.np(alloc.dtype)
            out_avals.append(jax.core.ShapedArray(shape, dtype))
            zero_shapes.append((shape, dtype))
    n_params = len(in_names)
    n_outs = len(out_avals)
    all_names = list(in_names) + list(out_names)
    if partition_name is not None:
        all_names.append(partition_name)
    donate = tuple(range(n_params, n_params + n_outs))

    def _body(*args):
        operands = list(args)
        if partition_name is not None:
            operands.append(b2j.partition_id_tensor())
        outs = b2j._bass_exec_p.bind(
            *operands,
            out_avals=tuple(out_avals),
            in_names=tuple(all_names),
            out_names=tuple(out_names),
            lowering_input_output_aliases=(),
            sim_require_finite=True,
            sim_require_nnan=True,
            nc=nc,
        )
        return tuple(outs)

    devices = jax.devices()[:NC]
    assert len(devices) >= NC, f"need {NC} devices, have {len(jax.devices())}"
    mesh = Mesh(_np.asarray(devices), ("core",))
    in_specs = (PartitionSpec("core"),) * (n_params + n_outs)
    out_specs = (PartitionSpec("core"),) * n_outs
    sharded = jax.jit(
        shard_map(_body, mesh=mesh, in_specs=in_specs, out_specs=out_specs,
                  check_rep=False),
        donate_argnums=donate,
        keep_unused=True,
    )

    def run(in_maps):
        concat_in = [
            np.concatenate([np.asarray(in_maps[c][nm]) for c in range(NC)], axis=0)
            for nm in in_names
        ]
        zeros = [np.zeros((NC * sh[0], *sh[1:]), dtp) for sh, dtp in zero_shapes]
        out_arrs = sharded(*concat_in, *zeros)
        return [
            {nm: np.asarray(out_arrs[i]).reshape(NC, *out_avals[i].shape)[c]
             for i, nm in enumerate(out_names)}
            for c in range(NC)
        ]

    _cache[key] = run
    return run


def kernel(loc_p, loc_t, grid):
    assert int(grid) == G
    loc_p = np.asarray(loc_p, dtype=np.float32)
    loc_t = np.asarray(loc_t, dtype=np.float32)
    n_p, n_t = loc_p.shape[0], loc_t.shape[0]

    P = _derived(loc_p)
    T = _derived(loc_t)

    # Pairs whose DELTA-dilated AABBs overlap; everything else is < 1e-14.
    ox = (P["x0"][:, None] <= T["x1"][None, :] + DELTA) & \
         (T["x0"][None, :] <= P["x1"][:, None] + DELTA)
    oy = (P["y0"][:, None] <= T["y1"][None, :] + DELTA) & \
         (T["y0"][None, :] <= P["y1"][:, None] + DELTA)
    idx = np.argwhere(ox & oy)
    if len(idx):
        idx = idx[~_sat_separated(P, T, idx[:, 0], idx[:, 1], float(DELTA))]

    # Round-robin pairs over cores; pad each core to U*128 with dummy pairs.
    per_core = [idx[c::NC] for c in range(NC)]
    U = max(1, -(-max(len(pc) for pc in per_core) // 128))

    basis = _basis()
    in_maps = []
    for c in range(NC):
        pc = per_core[c]
        lh = np.zeros((KR, U * 128), dtype=np.float32)
        if len(pc):
            lh[:24, :len(pc)] = _pair_coeffs(P, T, pc[:, 0], pc[:, 1])
        in_maps.append({"LHST": lh.astype(np.float16),
                        "BASIS": basis})

    try:
        res = _get_runner(U)(in_maps)
    except Exception:
        # Robust fallback: the stock (slower) dispatch path.
        from concourse.bass_utils import run_bass_kernel_spmd

        res = run_bass_kernel_spmd(
            _get_compiled(U), in_maps, core_ids=list(range(NC))).results

    out = np.zeros((n_p, n_t), dtype=np.float32)
    for c in range(NC):
        pc = per_core[c]
        if len(pc):
            vals = res[c]["OUT"].T.reshape(-1)[:len(pc)]
            out[pc[:, 0], pc[:, 1]] = vals
    return out


# revision 16
# speedup vs baseline: 9.3038x; 1.0245x over previous
"""Sparse PIoU (pixel-wise IoU) pairwise matrix kernel for Trainium2, 8 cores.

Math: for each pair (pred box n, target box m) the reference samples a 16x16
grid of the joint AABB and evaluates soft memberships
F = sigmoid(k(w/2-|A|)) * sigmoid(k(h/2-|B|)) per box, with (A, B) the pixel
offsets rotated into the box frame.  A and B are affine in the grid coords
(ug, uh), so the sigmoid args (s/2 -+ A) for all 256 pixels x 4 fields x
{P,Q} come from one K=24 matmul against a constant basis; since
P + Q = s >= 8 and k = 10, sigmoid(kP)*sigmoid(kQ) == sigmoid(k*min(P,Q)) to
machine precision, giving the |.| for free.

Sparsity: boxes are small (8..96 px) in a 640x640 field, so only ~8% of the
512x512 pairs have overlapping (2px-dilated) AABBs; every excluded pair has
true PIoU < 1e-14 (sigmoid tails at >= 2px separation), so the host computes
the dilated-AABB mask, packs ONLY surviving pairs one-per-partition into
units of 128, and scatters device results into a zero matrix.  ~21.3k pairs
-> 21 units/core instead of the dense 256 (n x m-chunk) units: ~12x less
device work, identical numerics on every surviving pair.

Device pipeline per unit u (128 pairs):
    PE  : 4 matmuls [24,128]x[24,512] -> PSUM [128, 2048] = P|Q sig args
    ACT : sig = Sigmoid(K * PQ)            [128, 2048] bf16 (one instruction)
    DVE : Gm  = sigP * sigQ                [128, 1024] (field memberships)
    DVE : Fp  = gA * gB,  accum -> S[u]    (fused product+reduce, STT)
    DVE : F12 = F1 * F2,  accum -> I[u]    (fused product+reduce, STT)
Epilogue (once): piou = I / (S - I + eps)  [128, U] -> one DMA out.

Host precomputes the per-pair coefficient slab directly in the transposed
fp16 layout the PE wants (lhsT [24, pairs]), eliminating the on-device
coefficient phase, PE transposes and stash copies of the dense kernel.

Dispatch uses a persistent jitted shard_map callable (cached per unit-count
U) so steady-state calls skip jax re-trace/re-lowering.
"""

import numpy as np

N = 512
M = 512
G = 16
NPIX = G * G
K_SLOPE = np.float32(10.0)
EPS = np.float32(1e-6)
NC = 8
KR = 32   # coefficient rows (24 used + 8 zero pad)
DELTA = np.float32(2.0)  # AABB dilation margin in px (excluded-pair PIoU < 1e-14)

_cache = {}


def _derived(b):
    # b: [K,5] float32 -> per-box derived quantities (all float32)
    cx, cy, w, h, t = (b[:, i].astype(np.float32) for i in range(5))
    c, s = np.cos(t).astype(np.float32), np.sin(t).astype(np.float32)
    hw = np.float32(0.5) * (w * np.abs(c) + h * np.abs(s))
    hh = np.float32(0.5) * (w * np.abs(s) + h * np.abs(c))
    return dict(
        cx=cx, cy=cy, ct=c, st=s,
        shw=np.float32(0.5) * w, shh=np.float32(0.5) * h,
        x0=cx - hw, x1=cx + hw, y0=cy - hh, y1=cy + hh,
    )


def _basis():
    # [KR, 2048] bf16 (values exact): P cols 0..1023, Q cols 1024..2047; field
    # f at cols f*256..(f+1)*256 uses rows 3f..3f+2 (P) / 12+3f.. (Q)
    # = (1, Ug, Uh).  Pixel p = h*G+g -> Ug[p]=u[g], Uh[p]=u[h].
    u = (np.arange(G, dtype=np.float32) + np.float32(0.5)) / np.float32(G)
    Ug = np.tile(u, G)
    Uh = np.repeat(u, G)
    bas = np.zeros((KR, 8 * NPIX), dtype=np.float32)
    for f in range(4):
        for blk, r0 in ((0, 0), (1, 12)):
            c0 = blk * 4 * NPIX + f * NPIX
            bas[r0 + 3 * f + 0, c0:c0 + NPIX] = 1.0
            bas[r0 + 3 * f + 1, c0:c0 + NPIX] = Ug
            bas[r0 + 3 * f + 2, c0:c0 + NPIX] = Uh
    import ml_dtypes

    return bas.astype(ml_dtypes.bfloat16)


def _sat_separated(P, T, n_idx, m_idx, margin):
    """True for pairs whose margin-dilated rotated boxes are disjoint
    (separating-axis test on the 4 edge normals).  At k=10, a separation
    margin of 2px bounds the true PIoU of excluded pairs below ~1e-12."""
    dcx = T["cx"][m_idx] - P["cx"][n_idx]
    dcy = T["cy"][m_idx] - P["cy"][n_idx]
    sep = np.zeros(n_idx.size, dtype=bool)
    for src in (0, 1):
        B1, i1 = (P, n_idx) if src == 0 else (T, m_idx)
        B2, i2 = (T, m_idx) if src == 0 else (P, n_idx)
        ct, st = B1["ct"][i1], B1["st"][i1]
        c2, s2 = B2["ct"][i2], B2["st"][i2]
        for ax in range(2):
            ux, uy = (ct, st) if ax == 0 else (-st, ct)
            e1 = B1["shw" if ax == 0 else "shh"][i1]
            e2 = (B2["shw"][i2] * np.abs(ux * c2 + uy * s2)
                  + B2["shh"][i2] * np.abs(-ux * s2 + uy * c2))
            sep |= np.abs(ux * dcx + uy * dcy) > e1 + e2 + margin
    return sep


def _pair_coeffs(P, T, n_idx, m_idx):
    """[24, npairs] float32 coefficient slab for the given (n, m) pairs.

    Row 3f+r holds the P-arg coeff of field f on basis fn r in (1, ug, uh);
    row 12+3f+r the Q-arg coeff.  Field order: A-pred, A-targ, B-pred, B-targ
    so that Gm pairs A1|A2 with B1|B2 and Fp pairs F1 with F2 downstream.
    """
    p = {k: v[n_idx] for k, v in P.items()}
    t = {k: v[m_idx] for k, v in T.items()}
    xmin = np.minimum(p["x0"], t["x0"])
    xmax = np.maximum(p["x1"], t["x1"])
    ymin = np.minimum(p["y0"], t["y0"])
    ymax = np.maximum(p["y1"], t["y1"])
    sx = xmax - xmin
    sy = ymax - ymin
    C = np.empty((24, n_idx.size), dtype=np.float32)
    for f, (b, ab) in enumerate(((p, "a"), (t, "a"), (p, "b"), (t, "b"))):
        dx0 = xmin - b["cx"]
        dy0 = ymin - b["cy"]
        if ab == "a":
            c0 = dx0 * b["ct"] + dy0 * b["st"]
            c1 = sx * b["ct"]
            c2 = sy * b["st"]
            half = b["shw"]
        else:
            c0 = dy0 * b["ct"] - dx0 * b["st"]
            c1 = -sx * b["st"]
            c2 = sy * b["ct"]
            half = b["shh"]
        C[3 * f + 0] = half - c0
        C[3 * f + 1] = -c1
        C[3 * f + 2] = -c2
        C[12 + 3 * f + 0] = half + c0
        C[12 + 3 * f + 1] = c1
        C[12 + 3 * f + 2] = c2
    return C


def _build_nc(U):
    from contextlib import ExitStack

    import concourse.bacc as bacc
    import concourse.tile as tile
    from concourse import mybir

    dt = mybir.dt
    op = mybir.AluOpType
    AF = mybir.ActivationFunctionType
    K = float(K_SLOPE)

    nc = bacc.Bacc(None, target_bir_lowering=False)
    LH_d = nc.declare_dram_parameter("LHST", [KR, U * 128], dt.float16, isOutput=False)
    BAS_d = nc.declare_dram_parameter("BASIS", [KR, 8 * NPIX], dt.bfloat16, isOutput=False)
    # Raw S|I accumulators; the tiny piou = I/(S-I+eps) division happens on
    # the host, which keeps the device tail to just the last STT + DMA.
    OUT_d = nc.declare_dram_parameter("OUT", [128, 2 * U], dt.float32, isOutput=True)

    with tile.TileContext(nc) as tc, ExitStack() as ctx:
        consts = ctx.enter_context(tc.tile_pool(name="consts", bufs=1))
        sigp = ctx.enter_context(tc.tile_pool(name="sigp", bufs=4))
        gmp = ctx.enter_context(tc.tile_pool(name="gmp", bufs=3))
        fpp = ctx.enter_context(tc.tile_pool(name="fpp", bufs=3))
        accp = ctx.enter_context(tc.tile_pool(name="accp", bufs=1))
        psum = ctx.enter_context(tc.tile_pool(name="psum", bufs=2, space="PSUM"))

        # Input DMAs fan out over engine queues (only sync/scalar/gpsimd can
        # trigger DMAs) and split into chunks, so transfers run concurrently
        # right after the post-preamble barrier and the first matmul's
        # operands land as early as possible.
        BAS = consts.tile([KR, 8 * NPIX], dt.bfloat16)
        LH = consts.tile([KR, U, 128], dt.float16)
        nchunk = min(4, U)
        bounds = [U * i // nchunk for i in range(nchunk + 1)]
        dmas = [(BAS[:, h * 512:(h + 1) * 512], BAS_d[:, h * 512:(h + 1) * 512])
                for h in range(4)]
        for i in range(nchunk):
            lo, hi = bounds[i], bounds[i + 1]
            dmas.append((LH[:, lo:hi, :].rearrange("p a b -> p (a b)"),
                         LH_d[:, lo * 128:hi * 128]))
        order = [0, 4, 1, 5, 2, 6, 3, 7]  # interleave BAS cols / LH chunks
        qs = [nc.sync, nc.scalar, nc.gpsimd]
        for j, k in enumerate(order[:len(dmas)]):
            out_ap, in_ap = dmas[k]
            qs[j % 3].dma_start(out=out_ap, in_=in_ap)

        SI = accp.tile([128, 2, U], dt.float32)
        Ssum = SI[:, 0, :]
        Isum = SI[:, 1, :]
        OUTv = OUT_d[:].rearrange("p (a b) -> p a b", a=2)

        # Warm the PE clock on a memset scratch tile: no DMA dependency, so
        # the ramp starts right after the preamble barrier.
        Wz = consts.tile([KR, 512], dt.bfloat16)
        nc.gpsimd.memset(Wz[:], 0.0)
        Tw = psum.tile([128, 8 * NPIX], dt.float32, tag="pq")
        nc.tensor.matmul(
            Tw[:, 0:512], Wz[:, 0:128], Wz[:], start=True, stop=True)

        # S|I columns DMA out in two chunks: all but the last unit while the
        # ACT stream is still running, the final column right at the end.
        U1 = U - 1 if U > 1 else U

        for u in range(U):
            PQ = psum.tile([128, 8 * NPIX], dt.float32, tag="pq")
            for h in range(4):
                nc.tensor.matmul(
                    PQ[:, h * 512:(h + 1) * 512],
                    LH[:, u, :],
                    BAS[:, h * 512:(h + 1) * 512],
                    start=True, stop=True)
            last = u == U - 1 and U > 1
            if not last:
                sig = sigp.tile([128, 8 * NPIX], dt.bfloat16, tag="sig")
                nc.scalar.activation(sig[:], PQ[:], AF.Sigmoid, 0.0, K)
                Gm = gmp.tile([128, 4 * NPIX], dt.bfloat16, tag="Gm")
                nc.vector.tensor_tensor(
                    Gm[:], sig[:, 0:1024], sig[:, 1024:2048], op.mult)
            else:
                # Final unit: sigmoid in two field-halves (A then B) so the
                # closing DVE chain starts one ACT-half earlier.  PQ viewed
                # [128, 2, 1024]: [:, :, 0:512] = A-fields' P|Q cols.
                PQv = PQ[:].rearrange("p (a b) -> p a b", a=2)
                Gm = gmp.tile([128, 4 * NPIX], dt.bfloat16, tag="Gm")
                for fh in range(2):
                    sig = sigp.tile([128, 8 * NPIX], dt.bfloat16, tag="sig")
                    sigv = sig[:, 0:1024].rearrange("p (a b) -> p a b", a=2)
                    nc.scalar.activation(
                        sigv, PQv[:, :, fh * 512:(fh + 1) * 512],
                        AF.Sigmoid, 0.0, K)
                    nc.vector.tensor_tensor(
                        Gm[:, fh * 512:(fh + 1) * 512],
                        sig[:, 0:512], sig[:, 512:1024], op.mult)
            Fp = fpp.tile([128, 2 * NPIX], dt.bfloat16, tag="Fp")
            nc.vector.scalar_tensor_tensor(
                Fp[:], Gm[:, 0:512], 1.0, Gm[:, 512:1024], op.mult, op.mult,
                accum_out=Ssum[:, u:u + 1])
            F12 = fpp.tile([128, NPIX], dt.bfloat16, tag="F12")
            nc.vector.scalar_tensor_tensor(
                F12[:], Fp[:, 0:NPIX], 1.0, Fp[:, NPIX:2 * NPIX], op.mult, op.mult,
                accum_out=Isum[:, u:u + 1])
            if u == U1 - 1 and U1 < U:
                nc.sync.dma_start(out=OUTv[:, :, 0:U1], in_=SI[:, :, 0:U1])

        if U1 < U:
            nc.gpsimd.dma_start(out=OUTv[:, :, U1:U], in_=SI[:, :, U1:U])
        else:
            nc.sync.dma_start(out=OUTv[:], in_=SI[:])

    nc.finalize()
    return nc


def _get_compiled(U):
    key = ("nc", U)
    if key not in _cache:
        _cache[key] = _build_nc(U)
    return _cache[key]


def _get_runner(U):
    """Persistent jitted shard_map callable (cached per unit count U)."""
    key = ("runner", U)
    if key in _cache:
        return _cache[key]

    import jax
    import numpy as _np
    from jax.experimental.shard_map import shard_map
    from jax.sharding import Mesh, PartitionSpec

    import concourse.bass2jax as b2j
    from concourse import mybir

    nc = _get_compiled(U)
    b2j.install_neuronx_cc_hook()
    partition_name = nc.partition_id_tensor.name if nc.partition_id_tensor else None

    in_names, out_names, out_avals, zero_shapes = [], [], [], []
    for alloc in nc.m.functions[0].allocations:
        if not isinstance(alloc, mybir.MemoryLocationSet):
            continue
        name = alloc.memorylocations[0].name
        if alloc.kind == "ExternalInput":
            if name != partition_name:
                in_names.append(name)
        elif alloc.kind == "ExternalOutput":
            out_names.append(name)
            shape = tuple(alloc.tensor_shape)
            dtype = mybir.dt.np(alloc.dtype)
            out_avals.append(jax.core.ShapedArray(shape, dtype))
            zero_shapes.append((shape, dtype))
    n_params = len(in_names)
    n_outs = len(out_avals)
    all_names = list(in_names) + list(out_names)
    if partition_name is not None:
        all_names.append(partition_name)
    donate = tuple(range(n_params, n_params + n_outs))

    def _body(*args):
        operands = list(args)
        if partition_name is not None:
            operands.append(b2j.partition_id_tensor())
        outs = b2j._bass_exec_p.bind(
            *operands,
            out_avals=tuple(out_avals),
            in_names=tuple(all_names),
            out_names=tuple(out_names),
            lowering_input_output_aliases=(),
            sim_require_finite=True,
            sim_require_nnan=True,
            nc=nc,
        )
        return tuple(outs)

    devices = jax.devices()[:NC]
    assert len(devices) >= NC, f"need {NC} devices, have {len(jax.devices())}"
    mesh = Mesh(_np.asarray(devices), ("core",))
    in_specs = (PartitionSpec("core"),) * (n_params + n_outs)
    out_specs = (PartitionSpec("core"),) * n_outs
    sharded = jax.jit(
        shard_map(_body, mesh=mesh, in_specs=in_specs, out_specs=out_specs,
                  check_rep=False),
        donate_argnums=donate,
        keep_unused=True,
    )

    def run(in_maps):
        concat_in = [
            np.concatenate([np.asarray(in_maps[c][nm]) for c in range(NC)], axis=0)
            for nm in in_names
        ]
        zeros = [np.zeros((NC * sh[0], *sh[1:]), dtp) for sh, dtp in zero_shapes]
        out_arrs = sharded(*concat_in, *zeros)
        return [
            {nm: np.asarray(out_arrs[i]).reshape(NC, *out_avals[i].shape)[c]
             for i, nm in enumerate(out_names)}
            for c in range(NC)
        ]

    _cache[key] = run
    return run


def kernel(loc_p, loc_t, grid):
    assert int(grid) == G
    loc_p = np.asarray(loc_p, dtype=np.float32)
    loc_t = np.asarray(loc_t, dtype=np.float32)
    n_p, n_t = loc_p.shape[0], loc_t.shape[0]

    P = _derived(loc_p)
    T = _derived(loc_t)

    # Pairs whose DELTA-dilated AABBs overlap; everything else is < 1e-14.
    ox = (P["x0"][:, None] <= T["x1"][None, :] + DELTA) & \
         (T["x0"][None, :] <= P["x1"][:, None] + DELTA)
    oy = (P["y0"][:, None] <= T["y1"][None, :] + DELTA) & \
         (T["y0"][None, :] <= P["y1"][:, None] + DELTA)
    idx = np.argwhere(ox & oy)
    if len(idx):
        idx = idx[~_sat_separated(P, T, idx[:, 0], idx[:, 1], float(DELTA))]

    # Round-robin pairs over cores; pad each core to U*128 with dummy pairs.
    per_core = [idx[c::NC] for c in range(NC)]
    U = max(1, -(-max(len(pc) for pc in per_core) // 128))

    basis = _basis()
    in_maps = []
    for c in range(NC):
        pc = per_core[c]
        lh = np.zeros((KR, U * 128), dtype=np.float32)
        if len(pc):
            lh[:24, :len(pc)] = _pair_coeffs(P, T, pc[:, 0], pc[:, 1])
        in_maps.append({"LHST": lh.astype(np.float16),
                        "BASIS": basis})

    try:
        res = _get_runner(U)(in_maps)
    except Exception:
        # Robust fallback: the stock (slower) dispatch path.
        from concourse.bass_utils import run_bass_kernel_spmd

        res = run_bass_kernel_spmd(
            _get_compiled(U), in_maps, core_ids=list(range(NC))).results

    out = np.zeros((n_p, n_t), dtype=np.float32)
    for c in range(NC):
        pc = per_core[c]
        if len(pc):
            si = res[c]["OUT"]  # [128, 2U]: S cols then I cols
            S = si[:, :U].T.reshape(-1)[:len(pc)]
            I = si[:, U:].T.reshape(-1)[:len(pc)]
            out[pc[:, 0], pc[:, 1]] = I / (S - I + EPS)
    return out


# revision 17
# speedup vs baseline: 9.3154x; 1.0012x over previous
"""Sparse PIoU (pixel-wise IoU) pairwise matrix kernel for Trainium2, 8 cores.

Math: for each pair (pred box n, target box m) the reference samples a 16x16
grid of the joint AABB and evaluates soft memberships
F = sigmoid(k(w/2-|A|)) * sigmoid(k(h/2-|B|)) per box, with (A, B) the pixel
offsets rotated into the box frame.  A and B are affine in the grid coords
(ug, uh), so the sigmoid args (s/2 -+ A) for all 256 pixels x 4 fields x
{P,Q} come from one K=24 matmul against a constant basis; since
P + Q = s >= 8 and k = 10, sigmoid(kP)*sigmoid(kQ) == sigmoid(k*min(P,Q)) to
machine precision, giving the |.| for free.

Sparsity: boxes are small (8..96 px) in a 640x640 field, so only ~8% of the
512x512 pairs have overlapping (2px-dilated) AABBs; every excluded pair has
true PIoU < 1e-14 (sigmoid tails at >= 2px separation), so the host computes
the dilated-AABB mask, packs ONLY surviving pairs one-per-partition into
units of 128, and scatters device results into a zero matrix.  ~21.3k pairs
-> 21 units/core instead of the dense 256 (n x m-chunk) units: ~12x less
device work, identical numerics on every surviving pair.

Device pipeline per unit u (128 pairs):
    PE  : 4 matmuls [24,128]x[24,512] -> PSUM [128, 2048] = P|Q sig args
    ACT : sig = Sigmoid(K * PQ)            [128, 2048] bf16 (one instruction)
    DVE : Gm  = sigP * sigQ                [128, 1024] (field memberships)
    DVE : Fp  = gA * gB,  accum -> S[u]    (fused product+reduce, STT)
    DVE : F12 = F1 * F2,  accum -> I[u]    (fused product+reduce, STT)
Epilogue (once): piou = I / (S - I + eps)  [128, U] -> one DMA out.

Host precomputes the per-pair coefficient slab directly in the transposed
fp16 layout the PE wants (lhsT [24, pairs]), eliminating the on-device
coefficient phase, PE transposes and stash copies of the dense kernel.

Dispatch uses a persistent jitted shard_map callable (cached per unit-count
U) so steady-state calls skip jax re-trace/re-lowering.
"""

import numpy as np

N = 512
M = 512
G = 16
NPIX = G * G
K_SLOPE = np.float32(10.0)
EPS = np.float32(1e-6)
NC = 8
KR = 32   # coefficient rows (24 used + 8 zero pad)
DELTA = np.float32(2.0)  # AABB dilation margin in px (excluded-pair PIoU < 1e-14)

_cache = {}


def _derived(b):
    # b: [K,5] float32 -> per-box derived quantities (all float32)
    cx, cy, w, h, t = (b[:, i].astype(np.float32) for i in range(5))
    c, s = np.cos(t).astype(np.float32), np.sin(t).astype(np.float32)
    hw = np.float32(0.5) * (w * np.abs(c) + h * np.abs(s))
    hh = np.float32(0.5) * (w * np.abs(s) + h * np.abs(c))
    return dict(
        cx=cx, cy=cy, ct=c, st=s,
        shw=np.float32(0.5) * w, shh=np.float32(0.5) * h,
        x0=cx - hw, x1=cx + hw, y0=cy - hh, y1=cy + hh,
    )


def _basis():
    # [KR, 2048] bf16 (values exact): P cols 0..1023, Q cols 1024..2047; field
    # f at cols f*256..(f+1)*256 uses rows 3f..3f+2 (P) / 12+3f.. (Q)
    # = (1, Ug, Uh).  Pixel p = h*G+g -> Ug[p]=u[g], Uh[p]=u[h].
    u = (np.arange(G, dtype=np.float32) + np.float32(0.5)) / np.float32(G)
    Ug = np.tile(u, G)
    Uh = np.repeat(u, G)
    bas = np.zeros((KR, 8 * NPIX), dtype=np.float32)
    for f in range(4):
        for blk, r0 in ((0, 0), (1, 12)):
            c0 = blk * 4 * NPIX + f * NPIX
            bas[r0 + 3 * f + 0, c0:c0 + NPIX] = 1.0
            bas[r0 + 3 * f + 1, c0:c0 + NPIX] = Ug
            bas[r0 + 3 * f + 2, c0:c0 + NPIX] = Uh
    import ml_dtypes

    return bas.astype(ml_dtypes.bfloat16)


def _sat_separated(P, T, n_idx, m_idx, margin):
    """True for pairs whose margin-dilated rotated boxes are disjoint
    (separating-axis test on the 4 edge normals).  At k=10, a separation
    margin of 2px bounds the true PIoU of excluded pairs below ~1e-12."""
    dcx = T["cx"][m_idx] - P["cx"][n_idx]
    dcy = T["cy"][m_idx] - P["cy"][n_idx]
    sep = np.zeros(n_idx.size, dtype=bool)
    for src in (0, 1):
        B1, i1 = (P, n_idx) if src == 0 else (T, m_idx)
        B2, i2 = (T, m_idx) if src == 0 else (P, n_idx)
        ct, st = B1["ct"][i1], B1["st"][i1]
        c2, s2 = B2["ct"][i2], B2["st"][i2]
        for ax in range(2):
            ux, uy = (ct, st) if ax == 0 else (-st, ct)
            e1 = B1["shw" if ax == 0 else "shh"][i1]
            e2 = (B2["shw"][i2] * np.abs(ux * c2 + uy * s2)
                  + B2["shh"][i2] * np.abs(-ux * s2 + uy * c2))
            sep |= np.abs(ux * dcx + uy * dcy) > e1 + e2 + margin
    return sep


def _pair_coeffs(P, T, n_idx, m_idx):
    """[24, npairs] float32 coefficient slab for the given (n, m) pairs.

    Row 3f+r holds the P-arg coeff of field f on basis fn r in (1, ug, uh);
    row 12+3f+r the Q-arg coeff.  Field order: A-pred, A-targ, B-pred, B-targ
    so that Gm pairs A1|A2 with B1|B2 and Fp pairs F1 with F2 downstream.
    """
    p = {k: v[n_idx] for k, v in P.items()}
    t = {k: v[m_idx] for k, v in T.items()}
    xmin = np.minimum(p["x0"], t["x0"])
    xmax = np.maximum(p["x1"], t["x1"])
    ymin = np.minimum(p["y0"], t["y0"])
    ymax = np.maximum(p["y1"], t["y1"])
    sx = xmax - xmin
    sy = ymax - ymin
    C = np.empty((24, n_idx.size), dtype=np.float32)
    for f, (b, ab) in enumerate(((p, "a"), (t, "a"), (p, "b"), (t, "b"))):
        dx0 = xmin - b["cx"]
        dy0 = ymin - b["cy"]
        if ab == "a":
            c0 = dx0 * b["ct"] + dy0 * b["st"]
            c1 = sx * b["ct"]
            c2 = sy * b["st"]
            half = b["shw"]
        else:
            c0 = dy0 * b["ct"] - dx0 * b["st"]
            c1 = -sx * b["st"]
            c2 = sy * b["ct"]
            half = b["shh"]
        C[3 * f + 0] = half - c0
        C[3 * f + 1] = -c1
        C[3 * f + 2] = -c2
        C[12 + 3 * f + 0] = half + c0
        C[12 + 3 * f + 1] = c1
        C[12 + 3 * f + 2] = c2
    return C


def _build_nc(U):
    from contextlib import ExitStack

    import concourse.bacc as bacc
    import concourse.tile as tile
    from concourse import mybir

    dt = mybir.dt
    op = mybir.AluOpType
    AF = mybir.ActivationFunctionType
    K = float(K_SLOPE)

    nc = bacc.Bacc(None, target_bir_lowering=False)
    LH_d = nc.declare_dram_parameter("LHST", [KR, U * 128], dt.float16, isOutput=False)
    BAS_d = nc.declare_dram_parameter("BASIS", [KR, 8 * NPIX], dt.bfloat16, isOutput=False)
    # Raw S|I accumulators; the tiny piou = I/(S-I+eps) division happens on
    # the host, which keeps the device tail to just the last STT + DMA.
    OUT_d = nc.declare_dram_parameter("OUT", [128, 2 * U], dt.float32, isOutput=True)

    with tile.TileContext(nc) as tc, ExitStack() as ctx:
        consts = ctx.enter_context(tc.tile_pool(name="consts", bufs=1))
        sigp = ctx.enter_context(tc.tile_pool(name="sigp", bufs=4))
        gmp = ctx.enter_context(tc.tile_pool(name="gmp", bufs=3))
        fpp = ctx.enter_context(tc.tile_pool(name="fpp", bufs=3))
        accp = ctx.enter_context(tc.tile_pool(name="accp", bufs=1))
        psum = ctx.enter_context(tc.tile_pool(name="psum", bufs=2, space="PSUM"))

        # Warm the PE clock on a memset scratch tile: no DMA dependency, so
        # the ramp starts right after the preamble barrier.  Emitted before
        # the gpsimd DMA trigger so it isn't queued behind it.
        Wz = consts.tile([KR, 512], dt.bfloat16)
        nc.gpsimd.memset(Wz[:], 0.0)
        Tw = psum.tile([128, 8 * NPIX], dt.float32, tag="pq")
        nc.tensor.matmul(
            Tw[:, 0:512], Wz[:, 0:128], Wz[:], start=True, stop=True)

        # Input DMAs fan out over the sync + gpsimd queues in chunks, so
        # transfers run concurrently right after the post-preamble barrier
        # and the first matmuls' operands land as early as possible.  The
        # scalar queue stays DMA-free: a DMA trigger there makes walrus
        # re-emit the sigmoid ACT_TABLE_LOAD, delaying the first ACTIVATE.
        BAS = consts.tile([KR, 8 * NPIX], dt.bfloat16)
        LH = consts.tile([KR, U, 128], dt.float16)
        if U > 2:
            bounds = [0, 2] + [2 + (U - 2) * i // 3 for i in (1, 2, 3)]
        else:
            bounds = [0, U]
        lh_chunks = [
            (LH[:, lo:hi, :].rearrange("p a b -> p (a b)"),
             LH_d[:, lo * 128:hi * 128])
            for lo, hi in zip(bounds[:-1], bounds[1:])]
        bas_chunks = [
            (BAS[:, h * 1024:(h + 1) * 1024], BAS_d[:, h * 1024:(h + 1) * 1024])
            for h in range(2)]
        for q, chunks in ((nc.sync, [bas_chunks[0]] + lh_chunks[0::2]),
                          (nc.gpsimd, [bas_chunks[1]] + lh_chunks[1::2])):
            for out_ap, in_ap in chunks:
                q.dma_start(out=out_ap, in_=in_ap)

        SI = accp.tile([128, 2, U], dt.float32)
        Ssum = SI[:, 0, :]
        Isum = SI[:, 1, :]
        OUTv = OUT_d[:].rearrange("p (a b) -> p a b", a=2)

        # S|I columns DMA out in two chunks: all but the last unit while the
        # ACT stream is still running, the final column right at the end.
        U1 = U - 1 if U > 1 else U

        for u in range(U):
            PQ = psum.tile([128, 8 * NPIX], dt.float32, tag="pq")
            for h in range(4):
                nc.tensor.matmul(
                    PQ[:, h * 512:(h + 1) * 512],
                    LH[:, u, :],
                    BAS[:, h * 512:(h + 1) * 512],
                    start=True, stop=True)
            last = u == U - 1 and U > 1
            if not last:
                sig = sigp.tile([128, 8 * NPIX], dt.bfloat16, tag="sig")
                nc.scalar.activation(sig[:], PQ[:], AF.Sigmoid, 0.0, K)
                Gm = gmp.tile([128, 4 * NPIX], dt.bfloat16, tag="Gm")
                nc.vector.tensor_tensor(
                    Gm[:], sig[:, 0:1024], sig[:, 1024:2048], op.mult)
            else:
                # Final unit: sigmoid in two field-halves (A then B) so the
                # closing DVE chain starts one ACT-half earlier.  PQ viewed
                # [128, 2, 1024]: [:, :, 0:512] = A-fields' P|Q cols.
                PQv = PQ[:].rearrange("p (a b) -> p a b", a=2)
                Gm = gmp.tile([128, 4 * NPIX], dt.bfloat16, tag="Gm")
                for fh in range(2):
                    sig = sigp.tile([128, 8 * NPIX], dt.bfloat16, tag="sig")
                    sigv = sig[:, 0:1024].rearrange("p (a b) -> p a b", a=2)
                    nc.scalar.activation(
                        sigv, PQv[:, :, fh * 512:(fh + 1) * 512],
                        AF.Sigmoid, 0.0, K)
                    nc.vector.tensor_tensor(
                        Gm[:, fh * 512:(fh + 1) * 512],
                        sig[:, 0:512], sig[:, 512:1024], op.mult)
            Fp = fpp.tile([128, 2 * NPIX], dt.bfloat16, tag="Fp")
            nc.vector.scalar_tensor_tensor(
                Fp[:], Gm[:, 0:512], 1.0, Gm[:, 512:1024], op.mult, op.mult,
                accum_out=Ssum[:, u:u + 1])
            F12 = fpp.tile([128, NPIX], dt.bfloat16, tag="F12")
            nc.vector.scalar_tensor_tensor(
                F12[:], Fp[:, 0:NPIX], 1.0, Fp[:, NPIX:2 * NPIX], op.mult, op.mult,
                accum_out=Isum[:, u:u + 1])
            if u == U1 - 1 and U1 < U:
                nc.sync.dma_start(out=OUTv[:, :, 0:U1], in_=SI[:, :, 0:U1])

        if U1 < U:
            nc.gpsimd.dma_start(out=OUTv[:, :, U1:U], in_=SI[:, :, U1:U])
        else:
            nc.sync.dma_start(out=OUTv[:], in_=SI[:])

    nc.finalize()
    return nc


def _get_compiled(U):
    key = ("nc", U)
    if key not in _cache:
        _cache[key] = _build_nc(U)
    return _cache[key]


def _get_runner(U):
    """Persistent jitted shard_map callable (cached per unit count U)."""
    key = ("runner", U)
    if key in _cache:
        return _cache[key]

    import jax
    import numpy as _np
    from jax.experimental.shard_map import shard_map
    from jax.sharding import Mesh, PartitionSpec

    import concourse.bass2jax as b2j
    from concourse import mybir

    nc = _get_compiled(U)
    b2j.install_neuronx_cc_hook()
    partition_name = nc.partition_id_tensor.name if nc.partition_id_tensor else None

    in_names, out_names, out_avals, zero_shapes = [], [], [], []
    for alloc in nc.m.functions[0].allocations:
        if not isinstance(alloc, mybir.MemoryLocationSet):
            continue
        name = alloc.memorylocations[0].name
        if alloc.kind == "ExternalInput":
            if name != partition_name:
                in_names.append(name)
        elif alloc.kind == "ExternalOutput":
            out_names.append(name)
            shape = tuple(alloc.tensor_shape)
            dtype = mybir.dt.np(alloc.dtype)
            out_avals.append(jax.core.ShapedArray(shape, dtype))
            zero_shapes.append((shape, dtype))
    n_params = len(in_names)
    n_outs = len(out_avals)
    all_names = list(in_names) + list(out_names)
    if partition_name is not None:
        all_names.append(partition_name)
    donate = tuple(range(n_params, n_params + n_outs))

    def _body(*args):
        operands = list(args)
        if partition_name is not None:
            operands.append(b2j.partition_id_tensor())
        outs = b2j._bass_exec_p.bind(
            *operands,
            out_avals=tuple(out_avals),
            in_names=tuple(all_names),
            out_names=tuple(out_names),
            lowering_input_output_aliases=(),
            sim_require_finite=True,
            sim_require_nnan=True,
            nc=nc,
        )
        return tuple(outs)

    devices = jax.devices()[:NC]
    assert len(devices) >= NC, f"need {NC} devices, have {len(jax.devices())}"
    mesh = Mesh(_np.asarray(devices), ("core",))
    in_specs = (PartitionSpec("core"),) * (n_params + n_outs)
    out_specs = (PartitionSpec("core"),) * n_outs
    sharded = jax.jit(
        shard_map(_body, mesh=mesh, in_specs=in_specs, out_specs=out_specs,
                  check_rep=False),
        donate_argnums=donate,
        keep_unused=True,
    )

    def run(in_maps):
        concat_in = [
            np.concatenate([np.asarray(in_maps[c][nm]) for c in range(NC)], axis=0)
            for nm in in_names
        ]
        zeros = [np.zeros((NC * sh[0], *sh[1:]), dtp) for sh, dtp in zero_shapes]
        out_arrs = sharded(*concat_in, *zeros)
        return [
            {nm: np.asarray(out_arrs[i]).reshape(NC, *out_avals[i].shape)[c]
             for i, nm in enumerate(out_names)}
            for c in range(NC)
        ]

    _cache[key] = run
    return run


def kernel(loc_p, loc_t, grid):
    assert int(grid) == G
    loc_p = np.asarray(loc_p, dtype=np.float32)
    loc_t = np.asarray(loc_t, dtype=np.float32)
    n_p, n_t = loc_p.shape[0], loc_t.shape[0]

    P = _derived(loc_p)
    T = _derived(loc_t)

    # Pairs whose DELTA-dilated AABBs overlap; everything else is < 1e-14.
    ox = (P["x0"][:, None] <= T["x1"][None, :] + DELTA) & \
         (T["x0"][None, :] <= P["x1"][:, None] + DELTA)
    oy = (P["y0"][:, None] <= T["y1"][None, :] + DELTA) & \
         (T["y0"][None, :] <= P["y1"][:, None] + DELTA)
    idx = np.argwhere(ox & oy)
    if len(idx):
        idx = idx[~_sat_separated(P, T, idx[:, 0], idx[:, 1], float(DELTA))]

    # Round-robin pairs over cores; pad each core to U*128 with dummy pairs.
    per_core = [idx[c::NC] for c in range(NC)]
    U = max(1, -(-max(len(pc) for pc in per_core) // 128))

    basis = _basis()
    in_maps = []
    for c in range(NC):
        pc = per_core[c]
        lh = np.zeros((KR, U * 128), dtype=np.float32)
        if len(pc):
            lh[:24, :len(pc)] = _pair_coeffs(P, T, pc[:, 0], pc[:, 1])
        in_maps.append({"LHST": lh.astype(np.float16),
                        "BASIS": basis})

    try:
        res = _get_runner(U)(in_maps)
    except Exception:
        # Robust fallback: the stock (slower) dispatch path.
        from concourse.bass_utils import run_bass_kernel_spmd

        res = run_bass_kernel_spmd(
            _get_compiled(U), in_maps, core_ids=list(range(NC))).results

    out = np.zeros((n_p, n_t), dtype=np.float32)
    for c in range(NC):
        pc = per_core[c]
        if len(pc):
            si = res[c]["OUT"]  # [128, 2U]: S cols then I cols
            S = si[:, :U].T.reshape(-1)[:len(pc)]
            I = si[:, U:].T.reshape(-1)[:len(pc)]
            out[pc[:, 0], pc[:, 1]] = I / (S - I + EPS)
    return out


# revision 23
# speedup vs baseline: 9.3859x; 1.0076x over previous
"""Sparse PIoU (pixel-wise IoU) pairwise matrix kernel for Trainium2, 8 cores.

Math: for each pair (pred box n, target box m) the reference samples a 16x16
grid of the joint AABB and evaluates soft memberships
F = sigmoid(k(w/2-|A|)) * sigmoid(k(h/2-|B|)) per box, with (A, B) the pixel
offsets rotated into the box frame.  A and B are affine in the grid coords
(ug, uh), so the sigmoid args (s/2 -+ A) for all 256 pixels x 4 fields x
{P,Q} come from one K=24 matmul against a constant basis; since
P + Q = s >= 8 and k = 10, sigmoid(kP)*sigmoid(kQ) == sigmoid(k*min(P,Q)) to
machine precision, giving the |.| for free.

Sparsity: boxes are small (8..96 px) in a 640x640 field, so only ~8% of the
512x512 pairs have overlapping (2px-dilated) AABBs; every excluded pair has
true PIoU < 1e-14 (sigmoid tails at >= 2px separation), so the host computes
the dilated-AABB mask, packs ONLY surviving pairs one-per-partition into
units of 128, and scatters device results into a zero matrix.  ~21.3k pairs
-> 21 units/core instead of the dense 256 (n x m-chunk) units: ~12x less
device work, identical numerics on every surviving pair.

Device pipeline per unit u (128 pairs):
    PE  : 4 matmuls [24,128]x[24,512] -> PSUM [128, 2048] = P|Q sig args
    ACT : sig = Sigmoid(K * PQ)            [128, 2048] bf16 (one instruction)
    DVE : Gm  = sigP * sigQ                [128, 1024] (field memberships)
    DVE : Fp  = gA * gB,  accum -> S[u]    (fused product+reduce, STT)
    DVE : F12 = F1 * F2,  accum -> I[u]    (fused product+reduce, STT)
Epilogue (once): piou = I / (S - I + eps)  [128, U] -> one DMA out.

Host precomputes the per-pair coefficient slab directly in the transposed
fp16 layout the PE wants (lhsT [24, pairs]), eliminating the on-device
coefficient phase, PE transposes and stash copies of the dense kernel.

Dispatch uses a persistent jitted shard_map callable (cached per unit-count
U) so steady-state calls skip jax re-trace/re-lowering.
"""

import numpy as np

N = 512
M = 512
G = 16
NPIX = G * G
K_SLOPE = np.float32(10.0)
EPS = np.float32(1e-6)
NC = 8
KR = 32   # coefficient rows (24 used + 8 zero pad)
DELTA = np.float32(2.0)  # AABB dilation margin in px (excluded-pair PIoU < 1e-14)

_cache = {}


def _derived(b):
    # b: [K,5] float32 -> per-box derived quantities (all float32)
    cx, cy, w, h, t = (b[:, i].astype(np.float32) for i in range(5))
    c, s = np.cos(t).astype(np.float32), np.sin(t).astype(np.float32)
    hw = np.float32(0.5) * (w * np.abs(c) + h * np.abs(s))
    hh = np.float32(0.5) * (w * np.abs(s) + h * np.abs(c))
    return dict(
        cx=cx, cy=cy, ct=c, st=s,
        shw=np.float32(0.5) * w, shh=np.float32(0.5) * h,
        x0=cx - hw, x1=cx + hw, y0=cy - hh, y1=cy + hh,
    )


def _basis():
    # [12, 1024] bf16 (values exact): field f at cols f*256..(f+1)*256 uses
    # rows 3f..3f+2 = (1, Ug, Uh).  Pixel p = h*G+g -> Ug[p]=u[g], Uh[p]=u[h].
    # The P and Q sigmoid-arg blocks share this basis (their coefficients are
    # two weight-sets against the same moving tensor).
    u = (np.arange(G, dtype=np.float32) + np.float32(0.5)) / np.float32(G)
    Ug = np.tile(u, G)
    Uh = np.repeat(u, G)
    bas = np.zeros((12, 4 * NPIX), dtype=np.float32)
    for f in range(4):
        c0 = f * NPIX
        bas[3 * f + 0, c0:c0 + NPIX] = 1.0
        bas[3 * f + 1, c0:c0 + NPIX] = Ug
        bas[3 * f + 2, c0:c0 + NPIX] = Uh
    import ml_dtypes

    return bas.astype(ml_dtypes.bfloat16)


def _sat_separated(P, T, n_idx, m_idx, margin):
    """True for pairs whose margin-dilated rotated boxes are disjoint
    (separating-axis test on the 4 edge normals).  At k=10, a separation
    margin of 2px bounds the true PIoU of excluded pairs below ~1e-12."""
    dcx = T["cx"][m_idx] - P["cx"][n_idx]
    dcy = T["cy"][m_idx] - P["cy"][n_idx]
    sep = np.zeros(n_idx.size, dtype=bool)
    for src in (0, 1):
        B1, i1 = (P, n_idx) if src == 0 else (T, m_idx)
        B2, i2 = (T, m_idx) if src == 0 else (P, n_idx)
        ct, st = B1["ct"][i1], B1["st"][i1]
        c2, s2 = B2["ct"][i2], B2["st"][i2]
        for ax in range(2):
            ux, uy = (ct, st) if ax == 0 else (-st, ct)
            e1 = B1["shw" if ax == 0 else "shh"][i1]
            e2 = (B2["shw"][i2] * np.abs(ux * c2 + uy * s2)
                  + B2["shh"][i2] * np.abs(-ux * s2 + uy * c2))
            sep |= np.abs(ux * dcx + uy * dcy) > e1 + e2 + margin
    return sep


def _pair_coeffs(P, T, n_idx, m_idx):
    """[24, npairs] float32 coefficient slab for the given (n, m) pairs.

    Row 3f+r holds the P-arg coeff of field f on basis fn r in (1, ug, uh);
    row 12+3f+r the Q-arg coeff.  Field order: A-pred, A-targ, B-pred, B-targ
    so that Gm pairs A1|A2 with B1|B2 and Fp pairs F1 with F2 downstream.
    """
    p = {k: v[n_idx] for k, v in P.items()}
    t = {k: v[m_idx] for k, v in T.items()}
    xmin = np.minimum(p["x0"], t["x0"])
    xmax = np.maximum(p["x1"], t["x1"])
    ymin = np.minimum(p["y0"], t["y0"])
    ymax = np.maximum(p["y1"], t["y1"])
    sx = xmax - xmin
    sy = ymax - ymin
    C = np.empty((24, n_idx.size), dtype=np.float32)
    for f, (b, ab) in enumerate(((p, "a"), (t, "a"), (p, "b"), (t, "b"))):
        dx0 = xmin - b["cx"]
        dy0 = ymin - b["cy"]
        if ab == "a":
            c0 = dx0 * b["ct"] + dy0 * b["st"]
            c1 = sx * b["ct"]
            c2 = sy * b["st"]
            half = b["shw"]
        else:
            c0 = dy0 * b["ct"] - dx0 * b["st"]
            c1 = -sx * b["st"]
            c2 = sy * b["ct"]
            half = b["shh"]
        C[3 * f + 0] = half - c0
        C[3 * f + 1] = -c1
        C[3 * f + 2] = -c2
        C[12 + 3 * f + 0] = half + c0
        C[12 + 3 * f + 1] = c1
        C[12 + 3 * f + 2] = c2
    return C


def _build_nc(U):
    from contextlib import ExitStack

    import concourse.bacc as bacc
    import concourse.tile as tile
    from concourse import mybir

    dt = mybir.dt
    op = mybir.AluOpType
    AF = mybir.ActivationFunctionType
    K = float(K_SLOPE)

    nc = bacc.Bacc(None, target_bir_lowering=False)
    LH_d = nc.declare_dram_parameter("LHST", [24, U * 128], dt.float16, isOutput=False)
    BAS_d = nc.declare_dram_parameter("BASIS", [12, 4 * NPIX], dt.bfloat16, isOutput=False)
    # Raw S|I accumulators; the tiny piou = I/(S-I+eps) division happens on
    # the host, which keeps the device tail to just the last STT + DMA.
    OUT_d = nc.declare_dram_parameter("OUT", [128, 2 * U], dt.float32, isOutput=True)

    with tile.TileContext(nc) as tc, ExitStack() as ctx:
        consts = ctx.enter_context(tc.tile_pool(name="consts", bufs=1))
        sigp = ctx.enter_context(tc.tile_pool(name="sigp", bufs=4))
        gmp = ctx.enter_context(tc.tile_pool(name="gmp", bufs=3))
        fpp = ctx.enter_context(tc.tile_pool(name="fpp", bufs=3))
        accp = ctx.enter_context(tc.tile_pool(name="accp", bufs=1))
        psum = ctx.enter_context(tc.tile_pool(name="psum", bufs=2, space="PSUM"))

        # Warm the PE clock on a memset scratch tile: no DMA dependency, so
        # the ramp starts right after the preamble barrier.  Emitted before
        # the gpsimd DMA triggers so it isn't queued behind them.
        Wz = consts.tile([12, 128], dt.bfloat16)
        nc.gpsimd.memset(Wz[:], 0.0)
        Tw = psum.tile([128, 8 * NPIX], dt.float32, tag="pq")
        nc.tensor.matmul(
            Tw[:, 0:128], Wz[:], Wz[:], start=True, stop=True)

        # Input DMAs fan out over the sync + gpsimd queues in chunks, so
        # transfers run concurrently right after the post-preamble barrier
        # and the first matmuls' operands land as early as possible.  The
        # scalar queue stays DMA-free: a DMA trigger there makes walrus
        # re-emit the sigmoid ACT_TABLE_LOAD, delaying the first ACTIVATE.
        BAS = consts.tile([12, 4 * NPIX], dt.bfloat16)
        LHP = consts.tile([12, U, 128], dt.float16)
        LHQ = consts.tile([12, U, 128], dt.float16)
        bounds = [0, min(2, U)] + ([U] if U > 2 else [])
        def lh_chunks(tile_, r0):
            return [(tile_[:, lo:hi, :].rearrange("p a b -> p (a b)"),
                     LH_d[r0:r0 + 12, lo * 128:hi * 128])
                    for lo, hi in zip(bounds[:-1], bounds[1:])]
        gp = [(BAS[:], BAS_d[:])] + lh_chunks(LHQ, 12)
        sy = lh_chunks(LHP, 0)
        for q, chunks in ((nc.gpsimd, gp), (nc.sync, sy)):
            for out_ap, in_ap in chunks:
                q.dma_start(out=out_ap, in_=in_ap)

        SI = accp.tile([128, 2, U], dt.float32)
        Ssum = SI[:, 0, :]
        Isum = SI[:, 1, :]
        OUTv = OUT_d[:].rearrange("p (a b) -> p a b", a=2)

        # S|I columns DMA out in two chunks: all but the last unit while the
        # ACT stream is still running, the final column right at the end.
        U1 = U - 1 if U > 1 else U

        for u in range(U):
            PQ = psum.tile([128, 8 * NPIX], dt.float32, tag="pq")
            for h in range(4):
                nc.tensor.matmul(
                    PQ[:, h * 512:(h + 1) * 512],
                    (LHP if h < 2 else LHQ)[:, u, :],
                    BAS[:, (h % 2) * 512:(h % 2 + 1) * 512],
                    start=True, stop=True)
            last = u == U - 1 and U > 1
            if not last:
                sig = sigp.tile([128, 8 * NPIX], dt.bfloat16, tag="sig")
                nc.scalar.activation(sig[:], PQ[:], AF.Sigmoid, 0.0, K)
                Gm = gmp.tile([128, 4 * NPIX], dt.bfloat16, tag="Gm")
                nc.vector.tensor_tensor(
                    Gm[:], sig[:, 0:1024], sig[:, 1024:2048], op.mult)
            else:
                # Final unit: sigmoid in two field-halves (A then B) so the
                # closing DVE chain starts one ACT-half earlier.  PQ viewed
                # [128, 2, 1024]: [:, :, 0:512] = A-fields' P|Q cols.
                PQv = PQ[:].rearrange("p (a b) -> p a b", a=2)
                Gm = gmp.tile([128, 4 * NPIX], dt.bfloat16, tag="Gm")
                for fh in range(2):
                    sig = sigp.tile([128, 8 * NPIX], dt.bfloat16, tag="sig")
                    sigv = sig[:, 0:1024].rearrange("p (a b) -> p a b", a=2)
                    nc.scalar.activation(
                        sigv, PQv[:, :, fh * 512:(fh + 1) * 512],
                        AF.Sigmoid, 0.0, K)
                    nc.vector.tensor_tensor(
                        Gm[:, fh * 512:(fh + 1) * 512],
                        sig[:, 0:512], sig[:, 512:1024], op.mult)
            Fp = fpp.tile([128, 2 * NPIX], dt.bfloat16, tag="Fp")
            nc.vector.scalar_tensor_tensor(
                Fp[:], Gm[:, 0:512], 1.0, Gm[:, 512:1024], op.mult, op.mult,
                accum_out=Ssum[:, u:u + 1])
            F12 = fpp.tile([128, NPIX], dt.bfloat16, tag="F12")
            nc.vector.scalar_tensor_tensor(
                F12[:], Fp[:, 0:NPIX], 1.0, Fp[:, NPIX:2 * NPIX], op.mult, op.mult,
                accum_out=Isum[:, u:u + 1])
            if u == U1 - 1 and U1 < U:
                nc.sync.dma_start(out=OUTv[:, :, 0:U1], in_=SI[:, :, 0:U1])

        if U1 < U:
            nc.sync.dma_start(out=OUTv[:, :, U1:U], in_=SI[:, :, U1:U])
        else:
            nc.sync.dma_start(out=OUTv[:], in_=SI[:])

    nc.finalize()
    return nc


def _get_compiled(U):
    key = ("nc", U)
    if key not in _cache:
        _cache[key] = _build_nc(U)
    return _cache[key]


def _get_runner(U):
    """Persistent jitted shard_map callable (cached per unit count U)."""
    key = ("runner", U)
    if key in _cache:
        return _cache[key]

    import jax
    import numpy as _np
    from jax.experimental.shard_map import shard_map
    from jax.sharding import Mesh, PartitionSpec

    import concourse.bass2jax as b2j
    from concourse import mybir

    nc = _get_compiled(U)
    b2j.install_neuronx_cc_hook()
    partition_name = nc.partition_id_tensor.name if nc.partition_id_tensor else None

    in_names, out_names, out_avals, zero_shapes = [], [], [], []
    for alloc in nc.m.functions[0].allocations:
        if not isinstance(alloc, mybir.MemoryLocationSet):
            continue
        name = alloc.memorylocations[0].name
        if alloc.kind == "ExternalInput":
            if name != partition_name:
                in_names.append(name)
        elif alloc.kind == "ExternalOutput":
            out_names.append(name)
            shape = tuple(alloc.tensor_shape)
            dtype = mybir.dt.np(alloc.dtype)
            out_avals.append(jax.core.ShapedArray(shape, dtype))
            zero_shapes.append((shape, dtype))
    n_params = len(in_names)
    n_outs = len(out_avals)
    all_names = list(in_names) + list(out_names)
    if partition_name is not None:
        all_names.append(partition_name)
    donate = tuple(range(n_params, n_params + n_outs))

    def _body(*args):
        operands = list(args)
        if partition_name is not None:
            operands.append(b2j.partition_id_tensor())
        outs = b2j._bass_exec_p.bind(
            *operands,
            out_avals=tuple(out_avals),
            in_names=tuple(all_names),
            out_names=tuple(out_names),
            lowering_input_output_aliases=(),
            sim_require_finite=True,
            sim_require_nnan=True,
            nc=nc,
        )
        return tuple(outs)

    devices = jax.devices()[:NC]
    assert len(devices) >= NC, f"need {NC} devices, have {len(jax.devices())}"
    mesh = Mesh(_np.asarray(devices), ("core",))
    in_specs = (PartitionSpec("core"),) * (n_params + n_outs)
    out_specs = (PartitionSpec("core"),) * n_outs
    sharded = jax.jit(
        shard_map(_body, mesh=mesh, in_specs=in_specs, out_specs=out_specs,
                  check_rep=False),
        donate_argnums=donate,
        keep_unused=True,
    )

    def run(in_maps):
        concat_in = [
            np.concatenate([np.asarray(in_maps[c][nm]) for c in range(NC)], axis=0)
            for nm in in_names
        ]
        zeros = [np.zeros((NC * sh[0], *sh[1:]), dtp) for sh, dtp in zero_shapes]
        out_arrs = sharded(*concat_in, *zeros)
        return [
            {nm: np.asarray(out_arrs[i]).reshape(NC, *out_avals[i].shape)[c]
             for i, nm in enumerate(out_names)}
            for c in range(NC)
        ]

    _cache[key] = run
    return run


def kernel(loc_p, loc_t, grid):
    assert int(grid) == G
    loc_p = np.asarray(loc_p, dtype=np.float32)
    loc_t = np.asarray(loc_t, dtype=np.float32)
    n_p, n_t = loc_p.shape[0], loc_t.shape[0]

    P = _derived(loc_p)
    T = _derived(loc_t)

    # Pairs whose DELTA-dilated AABBs overlap; everything else is < 1e-14.
    ox = (P["x0"][:, None] <= T["x1"][None, :] + DELTA) & \
         (T["x0"][None, :] <= P["x1"][:, None] + DELTA)
    oy = (P["y0"][:, None] <= T["y1"][None, :] + DELTA) & \
         (T["y0"][None, :] <= P["y1"][:, None] + DELTA)
    idx = np.argwhere(ox & oy)
    if len(idx):
        idx = idx[~_sat_separated(P, T, idx[:, 0], idx[:, 1], float(DELTA))]

    # Round-robin pairs over cores; pad each core to U*128 with dummy pairs.
    per_core = [idx[c::NC] for c in range(NC)]
    U = max(1, -(-max(len(pc) for pc in per_core) // 128))

    basis = _basis()
    in_maps = []
    for c in range(NC):
        pc = per_core[c]
        lh = np.zeros((24, U * 128), dtype=np.float32)
        if len(pc):
            lh[:, :len(pc)] = _pair_coeffs(P, T, pc[:, 0], pc[:, 1])
        in_maps.append({"LHST": lh.astype(np.float16),
                        "BASIS": basis})

    try:
        res = _get_runner(U)(in_maps)
    except Exception:
        # Robust fallback: the stock (slower) dispatch path.
        from concourse.bass_utils import run_bass_kernel_spmd

        res = run_bass_kernel_spmd(
            _get_compiled(U), in_maps, core_ids=list(range(NC))).results

    out = np.zeros((n_p, n_t), dtype=np.float32)
    for c in range(NC):
        pc = per_core[c]
        if len(pc):
            si = res[c]["OUT"]  # [128, 2U]: S cols then I cols
            S = si[:, :U].T.reshape(-1)[:len(pc)]
            I = si[:, U:].T.reshape(-1)[:len(pc)]
            out[pc[:, 0], pc[:, 1]] = I / (S - I + EPS)
    return out
